# revision 18
# baseline (speedup 1.0000x reference)
import sys
sys.path.insert(0, '/opt/trn_rl_repo')
import numpy as np
import ml_dtypes
from contextlib import ExitStack

import concourse.tile as tile
from concourse import bacc, mybir
import concourse.bass2jax as bass2jax
from concourse.ap import AP
from concourse.bass_utils import run_bass_kernel_spmd

BF = ml_dtypes.bfloat16
F32 = mybir.dt.float32
BF16 = mybir.dt.bfloat16
FP8 = mybir.dt.float8e4
AF = mybir.ActivationFunctionType
ALU = mybir.AluOpType
AX = mybir.AxisListType

TOKEN, STRIDE, NHEAD, EPS = 40, 4, 9, 1e-5
B, Bc, L = 32, 16, 2048
D, H, dk, S = 81, 9, 9, 512
FF, FC1, FC2, Y = 256, 512, 256, 4
NC = 8
BL = B // NC            # 4 local samples
TOKL = BL * S           # 2048 local tokens
NCH = TOKL // 128       # 16 chunks
CCH = (Bc * S) // 128   # 64 cali chunks
NTOT_BN = float(B * S)  # BN1-3 divisor
LP = L + 2 * TOKEN      # 2128 padded length
FC1L = FC1 // NC        # 64 fc1 cols per core

# ---------------- section offset tables ----------------
# Gather blob (bf16 elems, global offsets into the AllGathered flat [8*NG])
def _mk_alloc():
    st = [0]
    def a(n, align=2):
        st[0] = (st[0] + align - 1) // align * align
        o = st[0]; st[0] += n
        return o
    return a, st

_ga, _gst = _mk_alloc()
GOF = {
    'wqf':  _ga(D * D), 'wkf': _ga(D * D), 'wq2f': _ga(D * D),
    'wk2f': _ga(D * D), 'wv2f': _ga(D * D), 'wo1': _ga(D * D), 'wo2': _ga(D * D),
    'tvw':  _ga(D * D), 'beta1': _ga(D * D),
    'ff1w': _ga(D * FF),
    'ff2w0': _ga(128 * D), 'ff2w1': _ga(128 * D),
    'fc3w': _ga(128 * 2 * Y),
    # cvs/cms stored as fp8: half the bf16 footprint (offsets in bf16 units)
    'cvs':  _ga(128 * 4 * S // 2), 'cms': _ga(128 * 4 * S // 2),
    'pe':   _ga(128 * 4 * D), 'mask': _ga(128 * 4 * D),
    'cali': _ga(Bc * LP),
}
_GT = _gst[0]
NG = ((_GT + NC - 1) // NC + 1) // 2 * 2       # per-core slice len (even)
GTOT = NC * NG

# F section (f32 elems, offsets relative to F-section start)
_fa, _fst = _mk_alloc()
FOF = {
    'bsr':  _fa(128 * NCH),
    'BCn':  _fa(128 * 2), 'BCp': _fa(128 * 6),
    'g1s':  _fa(D), 'b1s': _fa(D),
    'ff1bs': _fa(128 * 2), 'g2s': _fa(128 * 2), 'b2s': _fa(128 * 2),
    'ff2bs': _fa(D), 'g3s': _fa(D), 'b3s': _fa(D),
    'tvb':  _fa(D),
    'fc1b': _fa(FC1L), 'bnf1g': _fa(FC1L), 'bnf1b': _fa(FC1L),
    'fc2b': _fa(FC2), 'bnf2g': _fa(FC2), 'bnf2b': _fa(FC2),
    'fc3b': _fa(Y),
}
NF = _fst[0]
F0 = NG                                        # bf16 offset of F section (even)
H0 = F0 + 2 * NF                               # bf16 offset of bf16 per-core section
_ha, _hst = _mk_alloc()
HOF = {'fc1ws': _ha(128 * 4 * FC1L), 'fc2ws': _ha(FC1L * FC2),
       'xpad': _ha(BL * LP)}
NH = _hst[0]
NTOT = H0 + NH
RG = [list(range(NC))]


def _build():
    nc = bacc.Bacc("TRN2", target_bir_lowering=False, debug=False,
                   enable_asserts=False, num_devices=NC)
    pin_d = nc.dram_tensor("pin", [1, NTOT], BF16, kind="ExternalInput")
    out_d = nc.dram_tensor("out", [B, Y], F32, kind="ExternalOutput")

    gin_d = nc.dram_tensor("gin", [1, NG], BF16)
    gath_d = nc.dram_tensor("gath", [NC, NG], BF16, addr_space="Shared")
    st1_in = nc.dram_tensor("st1_in", [1, 2 * D], F32)
    st1_out = nc.dram_tensor("st1_out", [1, 2 * D], F32, addr_space="Shared")
    st2_in = nc.dram_tensor("st2_in", [1, 2 * FF], F32)
    st2_out = nc.dram_tensor("st2_out", [1, 2 * FF], F32, addr_space="Shared")
    st3_in = nc.dram_tensor("st3_in", [1, 2 * D], F32)
    st3_out = nc.dram_tensor("st3_out", [1, 2 * D], F32, addr_space="Shared")
    fea_in = nc.dram_tensor("fea_in", [BL, S], F32)
    fea_out = nc.dram_tensor("fea_out", [B, S], F32, addr_space="Shared")
    ar2_in = nc.dram_tensor("ar2_in", [32, FC2], F32)
    ar2_out = nc.dram_tensor("ar2_out", [32, FC2], F32, addr_space="Shared")

    with tile.TileContext(nc) as tc, ExitStack() as ctx:
        pw = ctx.enter_context(tc.tile_pool(name="pw", bufs=1))     # weights/consts
        ps = ctx.enter_context(tc.tile_pool(name="ps", bufs=1))     # persistent acts
        pb = ctx.enter_context(tc.tile_pool(name="pb", bufs=2))     # rotating sbuf
        pp = ctx.enter_context(tc.tile_pool(name="pp", bufs=3, space="PSUM"))
        pacc = ctx.enter_context(tc.tile_pool(name="pacc", bufs=1, space="PSUM"))
        pt = ctx.enter_context(tc.tile_pool(name="pt", bufs=2, space="PSUM"))
        pscr = ctx.enter_context(tc.tile_pool(name="pscr", bufs=1))  # big scratch

        V = nc.vector; A_ = nc.scalar; T = nc.tensor; G = nc.gpsimd

        flat16 = pin_d[:].rearrange("a b -> (a b)")
        flat32 = flat16.bitcast(F32)
        gflat = gath_d[:].rearrange("k n -> (k n)")
        gh16 = gflat.tensor
        FB = F0 // 2   # f32-unit base of F section

        def gsl(name, n):
            return gflat[GOF[name]:GOF[name] + n]

        def fsl(name, n):
            return flat32[FB + FOF[name]:FB + FOF[name] + n]

        # ---- gather first: everything replicated rides the device fabric ----
        nc.sync.dma_start(gin_d[:], pin_d[:, 0:NG])
        nc.gpsimd.collective_compute("AllGather", ALU.bypass, replica_groups=RG,
                                     ins=[gin_d[:]], outs=[gath_d[:]])

        # ---- device-generated constants ----
        ident = pw.tile([128, 128], F32, tag="ident")
        V.memset(ident[:], 1.0)
        G.affine_select(ident[:], ident[:], [[-1, 128]], ALU.is_equal,
                        0.0, base=0, channel_multiplier=1)
        selT = pw.tile([dk, D * H], F32, tag="selT")
        V.memset(selT[:], 1.0)
        G.affine_select(selT[:], selT[:], [[-dk, H], [1, D]], ALU.is_equal,
                        0.0, base=0, channel_multiplier=-1)
        ones_r = pw.tile([1, 128], F32, tag="ones_r"); V.memset(ones_r[:], 1.0)
        ones128 = pw.tile([128, 1], F32, tag="ones128"); V.memset(ones128[:], 1.0)
        ones32 = pw.tile([32, 1], F32, tag="ones32"); V.memset(ones32[:], 1.0)
        epsc = pw.tile([128, 1], F32, tag="epsc"); V.memset(epsc[:], EPS)

        # ---- F-section loads (parallel with gather) ----
        def fload(name, p, cols):
            t = pw.tile([p, cols], F32, tag=name)
            nc.sync.dma_start(t[:], fsl(name, p * cols).rearrange("(p c) -> p c", c=cols))
            return t

        BCn = fload('BCn', 128, 2)
        BCp = fload('BCp', 128, 6)
        bsr = fload('bsr', 128, NCH)
        g1s = fload('g1s', D, 1); b1s = fload('b1s', D, 1)
        ff1bs = fload('ff1bs', 128, 2)
        g2s = fload('g2s', 128, 2); b2s = fload('b2s', 128, 2)
        ff2bs = fload('ff2bs', D, 1); g3s = fload('g3s', D, 1); b3s = fload('b3s', D, 1)
        tvb_r = fload('tvb', 1, D)
        fc1b = fload('fc1b', 1, FC1L)
        bnf1g = fload('bnf1g', 1, FC1L); bnf1b = fload('bnf1b', 1, FC1L)
        fc2b = fload('fc2b', 1, FC2)
        bnf2g = fload('bnf2g', 1, FC2); bnf2b = fload('bnf2b', 1, FC2)
        fc3b = fload('fc3b', 1, Y)

        # ---- head weights (per-core slices, bf16 -> f32) ----
        fc1ws = pw.tile([128, 4 * FC1L], F32, tag="fc1ws")
        fc2ws = pw.tile([FC1L, FC2], F32, tag="fc2ws")
        with tc.tile_pool(name="pld", bufs=1) as pld:
            t16 = pld.tile([128, 4 * FC1L], BF16, tag="fc1ws16")
            nc.sync.dma_start(t16[:], flat16[H0 + HOF['fc1ws']:H0 + HOF['fc1ws'] + 128 * 4 * FC1L]
                              .rearrange("(p c) -> p c", c=4 * FC1L))
            A_.activation(fc1ws[:], t16[:], AF.Copy)
            u16 = pld.tile([FC1L, FC2], BF16, tag="fc2ws16")
            nc.sync.dma_start(u16[:], flat16[H0 + HOF['fc2ws']:H0 + HOF['fc2ws'] + FC1L * FC2]
                              .rearrange("(p c) -> p c", c=FC2))
            A_.activation(fc2ws[:], u16[:], AF.Copy)

            # ---- gathered weights: bf16 -> f32 upcast ----
            def gload(name, p, cols, pool=pw):
                s16 = pld.tile([p, cols], BF16, tag=name + "16")
                nc.sync.dma_start(s16[:], gsl(name, p * cols).rearrange("(p c) -> p c", c=cols))
                t = pool.tile([p, cols], F32, tag=name)
                A_.activation(t[:], s16[:], AF.Copy)
                return t

            wqf = gload('wqf', D, D); wkf = gload('wkf', D, D)
            wq2f = gload('wq2f', D, D); wk2f = gload('wk2f', D, D); wv2f = gload('wv2f', D, D)
            wo1 = gload('wo1', D, D); wo2 = gload('wo2', D, D)
            tvw = gload('tvw', D, D); beta1 = gload('beta1', D, D)
            ff1w = gload('ff1w', D, FF)
            ff2w0 = gload('ff2w0', 128, D); ff2w1 = gload('ff2w1', 128, D)
            fc3w_s = gload('fc3w', 128, 2 * Y)
            pe_sb = gload('pe', 128, 4 * D)
            msk_sb = gload('mask', 128, 4 * D)

            # cvs/cms stay fp8 (feed matmuls directly; exp scale is tiny so fp8 noise is ~1e-3)
            g8 = gflat.bitcast(FP8)
            cvs = pw.tile([128, 4 * S], FP8, tag="cvs")
            nc.sync.dma_start(cvs[:], g8[2 * GOF['cvs']:2 * GOF['cvs'] + 128 * 4 * S]
                              .rearrange("(p c) -> p c", c=4 * S))
            cms = pw.tile([128, 4 * S], FP8, tag="cms")
            nc.sync.dma_start(cms[:], g8[2 * GOF['cms']:2 * GOF['cms'] + 128 * 4 * S]
                              .rearrange("(p c) -> p c", c=4 * S))

            # ---- E = exp(cwE * biasT) ----
            E = ps.tile([128, 4 * S], F32, tag="E")
            for ti in range(4):
                pbT = pp.tile([128, S], F32, tag="mm")
                for ui in range(4):
                    T.matmul(pbT[:], cvs[:, ui * S + ti * 128: ui * S + ti * 128 + 128],
                             cms[:, ui * S:(ui + 1) * S],
                             start=(ui == 0), stop=(ui == 3))
                A_.activation(E[:, ti * S:(ti + 1) * S], pbT[:], AF.Exp, scale=BCp[:, 1:2])

            # ---- tokenize x on device (overlapping-stride im2col DMA) ----
            h16 = flat16.tensor
            txT = ps.tile([D, TOKL], F32, tag="txT")
            for i in range(NCH):
                sc = i % 4
                b = i // 4
                x16 = pb.tile([128, D], BF16, tag="xtok16")
                nc.sync.dma_start(x16[:], AP(h16, H0 + HOF['xpad'] + LP * b + STRIDE * 128 * sc,
                                             [[STRIDE, 128], [1, D]]))
                r = pb.tile([128, D], F32, tag="ntok")
                V.tensor_scalar(r[:], x16[:], BCn[:, 0:1], BCn[:, 1:2], op0=ALU.mult, op1=ALU.add)
                V.tensor_tensor(r[:], r[:], msk_sb[:, sc * D:(sc + 1) * D], op=ALU.mult)
                V.tensor_tensor(r[:], r[:], pe_sb[:, sc * D:(sc + 1) * D], op=ALU.add)
                ptr = pt.tile([D, 128], F32, tag="tr")
                T.transpose(ptr[:], r[:], ident[:])
                A_.activation(txT[:, i * 128:(i + 1) * 128], ptr[:], AF.Copy)

        # ---- cali: tokenize (from gathered raw), project to cK/cV ----
        p2 = tc.tile_pool(name="p2", bufs=1)
        p2x = p2.__enter__()
        cK = p2x.tile([128, CCH * D], F32, tag="cK")
        cV = p2x.tile([128, CCH * D], F32, tag="cV")
        for j in range(CCH):
            sc = j % 4
            c = j // 4
            r16 = pb.tile([128, D], BF16, tag="ctok16")
            nc.sync.dma_start(r16[:], AP(gh16, GOF['cali'] + LP * c + STRIDE * 128 * sc,
                                         [[STRIDE, 128], [1, D]]))
            r = pb.tile([128, D], F32, tag="ntok")
            V.tensor_scalar(r[:], r16[:], BCn[:, 0:1], BCn[:, 1:2], op0=ALU.mult, op1=ALU.add)
            V.tensor_tensor(r[:], r[:], msk_sb[:, sc * D:(sc + 1) * D], op=ALU.mult)
            V.tensor_tensor(r[:], r[:], pe_sb[:, sc * D:(sc + 1) * D], op=ALU.add)
            ptr = pt.tile([D, 128], F32, tag="tr")
            T.transpose(ptr[:], r[:], ident[:])
            tcTs = pb.tile([D, 128], F32, tag="tcTs")
            A_.activation(tcTs[:], ptr[:], AF.Copy)
            pk = pp.tile([128, D], F32, tag="mm")
            T.matmul(pk[:], tcTs[:], wk2f[:])
            A_.activation(cK[:, j * D:(j + 1) * D], pk[:], AF.Copy)
            pv = pp.tile([128, D], F32, tag="mm")
            T.matmul(pv[:], tcTs[:], wv2f[:])
            A_.activation(cV[:, j * D:(j + 1) * D], pv[:], AF.Copy)

        # ---- Q2/K2/V2 token-major ----
        Q2 = p2x.tile([128, NCH * D], F32, tag="Q2")
        K2 = p2x.tile([128, NCH * D], F32, tag="K2")
        V2 = p2x.tile([128, NCH * D], F32, tag="V2")
        for i in range(NCH):
            lh = txT[:, i * 128:(i + 1) * 128]
            for w, dst in ((wq2f, Q2), (wk2f, K2), (wv2f, V2)):
                pj = pp.tile([128, D], F32, tag="mm")
                T.matmul(pj[:], lh, w[:])
                A_.activation(dst[:, i * D:(i + 1) * D], pj[:], AF.Copy)

        # ---- branch 2: banded batch attention (self + all cali columns) ----
        h2T = p2x.tile([D, TOKL], F32, tag="h2T")
        for i in range(NCH):
            sc = i % 4
            q = Q2[:, i * D:(i + 1) * D]
            sco = pb.tile([128, 17 * H], F32, tag="sco")
            prod = pb.tile([128, D], F32, tag="prod")
            V.tensor_tensor(prod[:], q, K2[:, i * D:(i + 1) * D], op=ALU.mult)
            V.tensor_reduce(sco[:, 0:H], prod[:].rearrange("p (h e) -> p h e", e=dk),
                            op=ALU.add, axis=AX.X)
            ckv = cK[:].rearrange("p (c f) -> p c f", f=4 * D)[:, :, sc * D:(sc + 1) * D]
            big = pb.tile([128, 16 * D], F32, tag="big")
            V.tensor_tensor(big[:].rearrange("p (c f) -> p c f", f=D),
                            ckv, q.unsqueeze(1).broadcast_to([128, 16, D]), op=ALU.mult)
            V.tensor_reduce(sco[:, H:].rearrange("p (c h) -> p c h", h=H),
                            big[:].rearrange("p (c h e) -> p c h e", h=H, e=dk),
                            op=ALU.add, axis=AX.X)
            esc = pb.tile([128, 17 * H], F32, tag="esc")
            A_.activation(esc[:], sco[:], AF.Exp, scale=1.0 / 3.0)
            den = pb.tile([128, H], F32, tag="den")
            V.tensor_reduce(den[:], esc[:].rearrange("p (k h) -> p h k", h=H),
                            op=ALU.add, axis=AX.X)
            rden = pb.tile([128, H], F32, tag="rden")
            V.reciprocal(rden[:], den[:])
            acc = pb.tile([128, D], F32, tag="acc")
            V.tensor_tensor(acc[:].rearrange("p (h e) -> p h e", e=dk),
                            V2[:, i * D:(i + 1) * D].rearrange("p (h e) -> p h e", e=dk),
                            esc[:, 0:H].unsqueeze(2).broadcast_to([128, H, dk]), op=ALU.mult)
            cvv = cV[:].rearrange("p (c f) -> p c f", f=4 * D)[:, :, sc * D:(sc + 1) * D]
            V.tensor_tensor(big[:].rearrange("p (c h e) -> p c h e", h=H, e=dk),
                            cvv.rearrange("p c (h e) -> p c h e", e=dk),
                            esc[:, H:].rearrange("p (c h) -> p c h", h=H).unsqueeze(3).broadcast_to([128, 16, H, dk]),
                            op=ALU.mult)
            accc = pb.tile([128, D], F32, tag="accc")
            V.tensor_reduce(accc[:], big[:].rearrange("p (c f) -> p f c", f=D),
                            op=ALU.add, axis=AX.X)
            V.tensor_tensor(acc[:], acc[:], accc[:], op=ALU.add)
            V.tensor_tensor(acc[:].rearrange("p (h e) -> p h e", e=dk),
                            acc[:].rearrange("p (h e) -> p h e", e=dk),
                            rden[:].unsqueeze(2).broadcast_to([128, H, dk]), op=ALU.mult)
            ptr = pt.tile([D, 128], F32, tag="tr")
            T.transpose(ptr[:], acc[:], ident[:])
            A_.activation(h2T[:, i * 128:(i + 1) * 128], ptr[:], AF.Copy)

        # ---- Wo2 -> h2o ; xh = tx + h2o ----
        h2oT = ps.tile([D, TOKL], F32, tag="h2oT")
        for n in range(4):
            p81 = pp.tile([D, S], F32, tag="mm")
            T.matmul(p81[:], wo2[:], h2T[:, n * S:(n + 1) * S])
            A_.activation(h2oT[:, n * S:(n + 1) * S], p81[:], AF.Copy)
        p2.__exit__(None, None, None)
        xhT = ps.tile([D, TOKL], F32, tag="xhT")
        V.tensor_tensor(xhT[:], txT[:], h2oT[:], op=ALU.add)

        # ---- branch 1 projections ----
        QsT = ps.tile([D, TOKL], F32, tag="QsT")
        KsT = ps.tile([D, TOKL], F32, tag="KsT")
        for n in range(4):
            p81 = pp.tile([D, S], F32, tag="mm")
            T.matmul(p81[:], wqf[:], xhT[:, n * S:(n + 1) * S])
            A_.activation(QsT[:, n * S:(n + 1) * S], p81[:], AF.Copy)
            p81b = pp.tile([D, S], F32, tag="mm")
            T.matmul(p81b[:], wkf[:], xhT[:, n * S:(n + 1) * S])
            A_.activation(KsT[:, n * S:(n + 1) * S], p81b[:], AF.Copy)
        Qstok = ps.tile([128, NCH * D], F32, tag="Qstok")
        for i in range(NCH):
            pj = pp.tile([128, D], F32, tag="mm")
            T.matmul(pj[:], xhT[:, i * 128:(i + 1) * 128], wqf[:])
            A_.activation(Qstok[:, i * D:(i + 1) * D], pj[:], AF.Copy)

        # ---- branch 1 attention ----
        attT = ps.tile([D, TOKL], F32, tag="attT")
        for b in range(BL):
            h1T_ps = pacc.tile([D, S], F32, tag="acc")
            for h in range(H):
                pqs = pp.tile([dk, S], F32, tag="mm")
                T.matmul(pqs[:], ident[0:D, h * dk:(h + 1) * dk], QsT[:, b * S:(b + 1) * S])
                Qsh = pb.tile([dk, S], F32, tag="Qsh")
                A_.activation(Qsh[:], pqs[:], AF.Copy)
                pks = pp.tile([dk, S], F32, tag="mm")
                T.matmul(pks[:], ident[0:D, h * dk:(h + 1) * dk], KsT[:, b * S:(b + 1) * S])
                Ksh = pb.tile([dk, S], F32, tag="Ksh")
                A_.activation(Ksh[:], pks[:], AF.Copy)
                hv = pacc.tile([dk, S], F32, tag="hv")
                hd = pacc.tile([1, S], F32, tag="hd")
                for k in range(4):
                    psc = pp.tile([128, S], F32, tag="mm")
                    T.matmul(psc[:], Ksh[:, k * 128:(k + 1) * 128], Qsh[:])
                    et = pb.tile([128, S], F32, tag="et")
                    A_.activation(et[:], psc[:], AF.Exp, scale=BCp[:, 0:1])
                    V.tensor_tensor(et[:], et[:], E[:, k * S:(k + 1) * S], op=ALU.mult)
                    T.matmul(hv[:], Qstok[:, (b * 4 + k) * D + h * dk:(b * 4 + k) * D + (h + 1) * dk],
                             et[:], start=(k == 0), stop=(k == 3))
                    T.matmul(hd[:], ones128[:], et[:], start=(k == 0), stop=(k == 3))
                rd = pb.tile([1, S], F32, tag="rec")
                V.reciprocal(rd[:], hd[:])
                prep = pp.tile([dk, S], F32, tag="mm")
                T.matmul(prep[:], ones_r[:, 0:dk], rd[:])
                reps = pb.tile([dk, S], F32, tag="reps")
                A_.activation(reps[:], prep[:], AF.Copy)
                vvn = pb.tile([dk, S], F32, tag="vvn")
                V.tensor_tensor(vvn[:], hv[:], reps[:], op=ALU.mult)
                T.matmul(h1T_ps[:], selT[:, h * D:(h + 1) * D], vvn[:],
                         start=(h == 0), stop=(h == 8))
            h1Tb = pb.tile([D, S], F32, tag="h1Tb")
            A_.activation(h1Tb[:], h1T_ps[:], AF.Copy)
            p81w = pp.tile([D, S], F32, tag="mm")
            T.matmul(p81w[:], wo1[:], h1Tb[:])
            V.tensor_scalar(attT[:, b * S:(b + 1) * S], p81w[:], BCp[:D, 3:4], None, op0=ALU.mult)
        V.tensor_scalar(h2oT[:], h2oT[:], BCp[:D, 2:3], None, op0=ALU.mult)
        V.tensor_tensor(attT[:], attT[:], h2oT[:], op=ALU.add)

        # ---- BN helper ----
        def bn_stats_ar(x_tiles, sti, sto, width):
            off = 0
            for t, p in x_tiles:
                s_ = pb.tile([p, 1], F32, tag="bnsum")
                V.tensor_reduce(s_[:], t[:], op=ALU.add, axis=AX.X)
                nc.sync.dma_start(sti[:, off:off + p].rearrange("a b -> (a b)"), s_[:])
                sq = pscr.tile([128, TOKL], F32, tag="scr")
                qs = pb.tile([p, 1], F32, tag="bnqs")
                A_.activation(sq[:p, 0:t.shape[1]], t[:], AF.Square)
                V.tensor_reduce(qs[:], sq[:p, 0:t.shape[1]], op=ALU.add, axis=AX.X)
                nc.sync.dma_start(sti[:, width + off:width + off + p].rearrange("a b -> (a b)"), qs[:])
                off += p
            nc.gpsimd.collective_compute("AllReduce", ALU.add, replica_groups=RG,
                                         ins=[sti[:]], outs=[sto[:]])
            out = []
            off = 0
            for t, p in x_tiles:
                st = pb.tile([p, 2], F32, tag="bnst")
                nc.sync.dma_start(st[:, 0:1], sto[:, off:off + p].rearrange("a b -> (a b)"))
                nc.sync.dma_start(st[:, 1:2], sto[:, width + off:width + off + p].rearrange("a b -> (a b)"))
                mean = pb.tile([p, 1], F32, tag="bnmean")
                V.tensor_scalar(mean[:], st[:, 0:1], 1.0 / NTOT_BN, None, op0=ALU.mult)
                var = pb.tile([p, 1], F32, tag="bnvar")
                V.tensor_scalar(var[:], st[:, 1:2], 1.0 / NTOT_BN, None, op0=ALU.mult)
                m2 = pb.tile([p, 1], F32, tag="bnm2")
                V.tensor_tensor(m2[:], mean[:], mean[:], op=ALU.mult)
                V.tensor_tensor(var[:], var[:], m2[:], op=ALU.subtract)
                std = pb.tile([p, 1], F32, tag="bnstd")
                A_.activation(std[:], var[:], AF.Sqrt, bias=epsc[:p, :])
                inv = pb.tile([p, 1], F32, tag="bninv")
                V.reciprocal(inv[:], std[:])
                out.append((mean, inv))
                off += p
            return out

        def bn_apply(dst, src, mean, inv, gg, bb, p):
            Ac = pb.tile([p, 1], F32, tag="bnA")
            V.tensor_tensor(Ac[:], inv[:], gg[:], op=ALU.mult)
            Bc_ = pb.tile([p, 1], F32, tag="bnB")
            V.tensor_tensor(Bc_[:], mean[:], Ac[:], op=ALU.mult)
            V.tensor_tensor(Bc_[:], bb[:], Bc_[:], op=ALU.subtract)
            V.tensor_scalar(dst[:], src[:], Ac[:], Bc_[:], op0=ALU.mult, op1=ALU.add)

        # ---- s1 = att + tx ; BN1 -> ma ----
        V.tensor_tensor(attT[:], attT[:], txT[:], op=ALU.add)
        (st1,) = bn_stats_ar([(attT, D)], st1_in, st1_out, D)
        pff = tc.tile_pool(name="pff", bufs=1)
        pffx = pff.__enter__()
        maT = pffx.tile([D, TOKL], F32, tag="maT")
        bn_apply(maT, attT, st1[0], st1[1], g1s, b1s, D)

        # ---- ff1 + BN2 ----
        f1a = pffx.tile([128, TOKL], F32, tag="f1a")
        f1b = pffx.tile([128, TOKL], F32, tag="f1b")
        for m, dst in ((0, f1a), (1, f1b)):
            for n in range(4):
                pw5 = pp.tile([128, S], F32, tag="mm")
                T.matmul(pw5[:], ff1w[:, m * 128:(m + 1) * 128], maT[:, n * S:(n + 1) * S])
                A_.activation(dst[:, n * S:(n + 1) * S], pw5[:], AF.Relu, bias=ff1bs[:, m:m + 1])
        stats2 = bn_stats_ar([(f1a, 128), (f1b, 128)], st2_in, st2_out, FF)
        bn_apply(f1a, f1a, stats2[0][0], stats2[0][1], g2s[:, 0:1], b2s[:, 0:1], 128)
        bn_apply(f1b, f1b, stats2[1][0], stats2[1][1], g2s[:, 1:2], b2s[:, 1:2], 128)

        # ---- ff2 ; s3 = ma + f2 ; BN3 -> res ----
        for n in range(4):
            pf2 = pp.tile([D, S], F32, tag="mm")
            T.matmul(pf2[:], ff2w0[:], f1a[:, n * S:(n + 1) * S], start=True, stop=False)
            T.matmul(pf2[:], ff2w1[:], f1b[:, n * S:(n + 1) * S], start=False, stop=True)
            f2c = pscr.tile([128, TOKL], F32, tag="scr")
            A_.activation(f2c[:D, 0:S], pf2[:], AF.Relu, bias=ff2bs[:])
            V.tensor_tensor(maT[:, n * S:(n + 1) * S], maT[:, n * S:(n + 1) * S], f2c[:D, 0:S], op=ALU.add)
        (st3,) = bn_stats_ar([(maT, D)], st3_in, st3_out, D)
        resT = pffx.tile([D, TOKL], F32, tag="resT")
        bn_apply(resT, maT, st3[0], st3[1], g3s, b3s, D)

        # ---- head: a, fea ----
        ptvb = pp.tile([128, D], F32, tag="mm")
        T.matmul(ptvb[:], ones_r[:], tvb_r[:])
        TVBr = pw.tile([128, D], F32, tag="TVBr")
        A_.activation(TVBr[:], ptvb[:], AF.Copy)
        feas = ps.tile([128, NCH], F32, tag="feas")
        for i in range(NCH):
            lh = resT[:, i * 128:(i + 1) * 128]
            ptv = pp.tile([128, D], F32, tag="mm")
            T.matmul(ptv[:], lh, tvw[:])
            tv = pb.tile([128, D], F32, tag="tv")
            V.tensor_tensor(tv[:], ptv[:], TVBr[:], op=ALU.add)
            pbt = pp.tile([128, D], F32, tag="mm")
            T.matmul(pbt[:], lh, beta1[:])
            eb = pb.tile([128, D], F32, tag="eb")
            ebs = pb.tile([128, 1], F32, tag="ebs")
            A_.activation(eb[:], pbt[:], AF.Exp)
            V.tensor_reduce(ebs[:], eb[:], op=ALU.add, axis=AX.X)
            rb = pb.tile([128, 1], F32, tag="rb")
            V.reciprocal(rb[:], ebs[:])
            V.tensor_tensor(tv[:], tv[:], eb[:], op=ALU.mult)
            av = pb.tile([128, 1], F32, tag="av")
            V.tensor_reduce(av[:], tv[:], op=ALU.add, axis=AX.X)
            V.tensor_tensor(av[:], av[:], rb[:], op=ALU.mult)
            V.tensor_scalar(av[:], av[:], BCp[:, 4:5], None, op0=ALU.mult)
            u = pb.tile([128, 1], F32, tag="u")
            V.tensor_scalar(u[:], bsr[:, i:i + 1], BCp[:, 5:6], None, op0=ALU.mult)
            V.tensor_tensor(feas[:, i:i + 1], av[:], u[:], op=ALU.add)
        pff.__exit__(None, None, None)
        nc.sync.dma_start(fea_in[:].rearrange("b s -> (b s)").rearrange("(j p) -> p j", p=128), feas[:])
        nc.gpsimd.collective_compute("AllGather", ALU.bypass, replica_groups=RG,
                                     ins=[fea_in[:]], outs=[fea_out[:]])

        # ---- sharded head: fc1 cols 64k..64k+64 local, fc2 partial + AllReduce ----
        with tc.tile_pool(name="ph", bufs=1) as ph:
            feaT = ph.tile([128, 4 * 32], F32, tag="feaT")
            for k_ in range(4):
                nc.sync.dma_start(feaT[:, k_ * 32:(k_ + 1) * 32],
                                  fea_out[:, k_ * 128:(k_ + 1) * 128].rearrange("b p -> p b"))
            ph1 = pacc.tile([32, FC1L], F32, tag="acc")
            for k_ in range(4):
                T.matmul(ph1[:], feaT[:, k_ * 32:(k_ + 1) * 32],
                         fc1ws[:, k_ * FC1L:(k_ + 1) * FC1L], start=(k_ == 0), stop=False)
            T.matmul(ph1[:], ones_r[:, 0:32], fc1b[:], start=False, stop=True)
            hh = ph.tile([32, FC1L], F32, tag="hh")
            A_.activation(hh[:], ph1[:], AF.Relu)

            def head_bn(xt, cols, gg, bb):
                pms = pp.tile([1, cols], F32, tag="mm")
                T.matmul(pms[:], ones32[:], xt[:])
                hsq = pscr.tile([128, TOKL], F32, tag="scr")
                V.tensor_tensor(hsq[0:32, 0:cols], xt[:], xt[:], op=ALU.mult)
                psq = pp.tile([1, cols], F32, tag="mm")
                T.matmul(psq[:], ones32[:], hsq[0:32, 0:cols])
                mean = ph.tile([1, FC2], F32, tag="hmean")
                V.tensor_scalar(mean[:, 0:cols], pms[:], 1.0 / 32.0, None, op0=ALU.mult)
                var = ph.tile([1, FC2], F32, tag="hvar")
                V.tensor_scalar(var[:, 0:cols], psq[:], 1.0 / 32.0, None, op0=ALU.mult)
                m2 = ph.tile([1, FC2], F32, tag="hm2")
                V.tensor_tensor(m2[:, 0:cols], mean[:, 0:cols], mean[:, 0:cols], op=ALU.mult)
                V.tensor_tensor(var[:, 0:cols], var[:, 0:cols], m2[:, 0:cols], op=ALU.subtract)
                std = ph.tile([1, FC2], F32, tag="hstd")
                A_.activation(std[:, 0:cols], var[:, 0:cols], AF.Sqrt, bias=epsc[0:1, :])
                inv = ph.tile([1, FC2], F32, tag="hinv")
                V.reciprocal(inv[:, 0:cols], std[:, 0:cols])
                Ar = ph.tile([1, FC2], F32, tag="hA")
                V.tensor_tensor(Ar[:, 0:cols], inv[:, 0:cols], gg[:], op=ALU.mult)
                Br = ph.tile([1, FC2], F32, tag="hB")
                V.tensor_tensor(Br[:, 0:cols], mean[:, 0:cols], Ar[:, 0:cols], op=ALU.mult)
                V.tensor_tensor(Br[:, 0:cols], bb[:], Br[:, 0:cols], op=ALU.subtract)
                pA = pp.tile([32, cols], F32, tag="mm")
                T.matmul(pA[:], ones_r[:, 0:32], Ar[:, 0:cols])
                pB = pp.tile([32, cols], F32, tag="mm")
                T.matmul(pB[:], ones_r[:, 0:32], Br[:, 0:cols])
                As_ = ph.tile([32, FC2], F32, tag="hAs")
                A_.activation(As_[:, 0:cols], pA[:], AF.Copy)
                Bs_ = ph.tile([32, FC2], F32, tag="hBs")
                A_.activation(Bs_[:, 0:cols], pB[:], AF.Copy)
                V.tensor_tensor(xt[:], xt[:], As_[:, 0:cols], op=ALU.mult)
                V.tensor_tensor(xt[:], xt[:], Bs_[:, 0:cols], op=ALU.add)

            head_bn(hh, FC1L, bnf1g, bnf1b)
            # fc2 partial: [32, 64] x [64, 256]
            ptk = pt.tile([FC1L, 32], F32, tag="tr")
            T.transpose(ptk[:], hh[:], ident[0:32, 0:32])
            hT = ph.tile([FC1L, 32], F32, tag="hT")
            A_.activation(hT[:], ptk[:], AF.Copy)
            ph2 = pacc.tile([32, FC2], F32, tag="acc")
            T.matmul(ph2[:], hT[:], fc2ws[:])
            p2s = ph.tile([32, FC2], F32, tag="p2s")
            A_.activation(p2s[:], ph2[:], AF.Copy)
            nc.sync.dma_start(ar2_in[:], p2s[:])
            nc.gpsimd.collective_compute("AllReduce", ALU.add, replica_groups=RG,
                                         ins=[ar2_in[:]], outs=[ar2_out[:]])
            gsum = ph.tile([32, FC2], F32, tag="gsum")
            nc.sync.dma_start(gsum[:], ar2_out[:])
            pbias = pp.tile([32, FC2], F32, tag="mm")
            T.matmul(pbias[:], ones_r[:, 0:32], fc2b[:])
            V.tensor_tensor(gsum[:], gsum[:], pbias[:], op=ALU.add)
            gh = ph.tile([32, FC2], F32, tag="gh")
            A_.activation(gh[:], gsum[:], AF.Relu)
            head_bn(gh, FC2, bnf2g, bnf2b)
            ph3 = pacc.tile([32, Y], F32, tag="acc")
            for k_ in range(2):
                ptk2 = pt.tile([128, 32], F32, tag="tr")
                T.transpose(ptk2[:], gh[:, k_ * 128:(k_ + 1) * 128], ident[0:32, 0:32])
                gTk = pb.tile([128, 32], F32, tag="gTk")
                A_.activation(gTk[:], ptk2[:], AF.Copy)
                T.matmul(ph3[:], gTk[:], fc3w_s[:, k_ * Y:(k_ + 1) * Y],
                         start=(k_ == 0), stop=False)
            T.matmul(ph3[:], ones_r[:, 0:32], fc3b[:], start=False, stop=True)
            osb = ph.tile([32, Y], F32, tag="osb")
            A_.activation(osb[:], ph3[:], AF.Tanh)
            nc.sync.dma_start(out_d[:], osb[:])
    nc.compile()
    return nc


# ---------------- cached PJRT dispatch ----------------
_PJRT_CACHE = {}
_orig_run_via_pjrt = bass2jax.run_bass_via_pjrt


def _cached_run_bass_via_pjrt(nc, in_maps, n_cores):
    try:
        import jax
        key = (id(nc), n_cores)
        ent = _PJRT_CACHE.get(key)
        if ent is None:
            bass2jax.install_neuronx_cc_hook()
            if nc.dbg_addr is not None:
                raise RuntimeError("dbg path not cached")
            partition_name = nc.partition_id_tensor.name if nc.partition_id_tensor else None
            in_names, out_names, out_avals, zero_shapes = [], [], [], []
            for alloc in nc.m.functions[0].allocations:
                if not isinstance(alloc, mybir.MemoryLocationSet):
                    continue
                name = alloc.memorylocations[0].name
                if alloc.kind == "ExternalInput":
                    if name != partition_name:
                        in_names.append(name)
                elif alloc.kind == "ExternalOutput":
                    out_names.append(name)
                    shape = tuple(alloc.tensor_shape)
                    dtype = mybir.dt.np(alloc.dtype)
                    out_avals.append(jax.core.ShapedArray(shape, dtype))
                    zero_shapes.append((shape, dtype))
            n_params = len(in_names)
            n_outs = len(out_avals)
            all_names = list(in_names) + out_names + ([partition_name] if partition_name else [])

            import jax.numpy as jnp

            def _body(*args):
                # out buffers created in-graph: the kernel writes every output
                # element, so no host-side donated zeros are needed (saves one
                # host->device array RPC per call).
                operands = list(args) + [jnp.zeros(s, d) for (s, d) in zero_shapes]
                if partition_name is not None:
                    operands.append(bass2jax.partition_id_tensor())
                outs = bass2jax._bass_exec_p.bind(
                    *operands, out_avals=tuple(out_avals), in_names=tuple(all_names),
                    out_names=tuple(out_names), lowering_input_output_aliases=(),
                    sim_require_finite=True, sim_require_nnan=True, nc=nc)
                return tuple(outs)

            devices = jax.devices()[:n_cores]
            mesh = bass2jax.Mesh(np.asarray(devices), ("core",))
            in_specs = (bass2jax.PartitionSpec("core"),) * n_params
            out_specs = (bass2jax.PartitionSpec("core"),) * n_outs
            sharded = jax.jit(
                bass2jax.shard_map(_body, mesh=mesh, in_specs=in_specs,
                                   out_specs=out_specs, check_rep=False),
                keep_unused=True)
            ent = (sharded, in_names, out_names, out_avals, zero_shapes)
            _PJRT_CACHE[key] = ent
        sharded, in_names, out_names, out_avals, zero_shapes = ent
        concat_in = [np.concatenate([np.asarray(m[nm]) for m in in_maps], axis=0)
                     for nm in in_names]
        out_arrs = sharded(*concat_in)
        return [
            {nm: np.asarray(out_arrs[i]).reshape(n_cores, *out_avals[i].shape)[c]
             for i, nm in enumerate(out_names)}
            for c in range(n_cores)
        ]
    except Exception:
        _PJRT_CACHE.pop((id(nc), n_cores), None)
        return _orig_run_via_pjrt(nc, in_maps, n_cores)


bass2jax.run_bass_via_pjrt = _cached_run_bass_via_pjrt


# ---------------- host packing ----------------
def _pe_mask_imgs():
    f = np.float32
    idx = np.arange(S)[:, None] * STRIDE + np.arange(D)[None, :]
    mask = ((idx >= TOKEN) & (idx < TOKEN + L)).astype(f)
    pos = np.arange(S, dtype=f)[:, None]
    div = np.exp(-np.log(f(10000.0)) * np.arange(0, D, 2, dtype=f) / D)
    ang = pos * div
    pe = np.zeros((S, D), dtype=f)
    pe[:, 0::2] = np.sin(ang)
    pe[:, 1::2] = np.cos(ang[:, : D // 2])
    img = lambda m: np.ascontiguousarray(m.reshape(4, 128, D).transpose(1, 0, 2)).reshape(128, 4 * D)
    return img(pe), img(mask)


_PE_IMG, _MASK_IMG = _pe_mask_imgs()


def _host_inputs(x, basel, cali_spec, Wq, Wk, Wq2, Wk2, Wv2, Cv, Wo1, Wo2,
                 corr_weight, h_weight, corr_map, g1, b1, ff1_w, ff1_b, g2, b2,
                 ff2_w, ff2_b, g3, b3, token_v_w, token_v_b, beta1, alpha1, alpha2,
                 fc1_w, fc1_b, bnf1_g, bnf1_b, fc2_w, fc2_b, bnf2_g, bnf2_b, fc3_w, fc3_b):
    f = np.float32
    x = np.asarray(x, f); basel = np.asarray(basel, f); cali_spec = np.asarray(cali_spec, f)

    # ---- gather blob (global, bf16) ----
    flat = lambda w: np.ascontiguousarray(np.asarray(w, f).transpose(1, 0, 2)).reshape(D, D)
    img128 = lambda m, c, w: np.ascontiguousarray(
        np.asarray(m, f).reshape(c, 128, w).transpose(1, 0, 2)).reshape(128, c * w)
    gbuf = np.zeros(GTOT, dtype=f)

    def gput(name, arr):
        a = np.asarray(arr, f).reshape(-1)
        gbuf[GOF[name]:GOF[name] + a.size] = a

    gput('wqf', flat(Wq)); gput('wkf', flat(Wk)); gput('wq2f', flat(Wq2))
    gput('wk2f', flat(Wk2)); gput('wv2f', flat(Wv2))
    gput('wo1', Wo1); gput('wo2', Wo2)
    gput('tvw', token_v_w); gput('beta1', beta1)
    gput('ff1w', ff1_w)
    gput('ff2w0', np.asarray(ff2_w, f)[0:128, :]); gput('ff2w1', np.asarray(ff2_w, f)[128:256, :])
    gput('fc3w', img128(fc3_w, 2, Y))
    gput('pe', _PE_IMG); gput('mask', _MASK_IMG)
    gput('cali', np.pad(cali_spec, ((0, 0), (TOKEN, TOKEN))))
    gblob = gbuf.astype(BF)
    np8 = ml_dtypes.float8_e4m3
    for nm, m in (('cvs', img128(Cv, 4, S)), ('cms', img128(np.asarray(corr_map, f).T, 4, S))):
        raw = np.frombuffer(m.astype(np8).tobytes(), dtype=BF)
        gblob[GOF[nm]:GOF[nm] + raw.size] = raw

    # ---- exact normalization scalars (reference semantics, host f32) ----
    xm = x[:, 20:-20].min()
    xs = np.abs((x[:, 20:-20] - xm).max())
    A = f(1.0) / xs
    Bn = -xm * A
    cw = f(np.asarray(corr_weight).reshape(-1)[0])
    hw = f(np.asarray(h_weight).reshape(-1)[0])
    a1 = f(np.asarray(alpha1).reshape(-1)[0])
    a2 = f(np.asarray(alpha2).reshape(-1)[0])
    bcn = np.broadcast_to(np.array([A, Bn], f), (128, 2))
    bcp = np.broadcast_to(np.array([(1.0 - cw) / 3.0, cw / np.sqrt(f(S)),
                                    hw, 1.0 - hw, a1, a2], f), (128, 6))

    xpad = np.pad(x, ((0, 0), (TOKEN, TOKEN)))        # [B, 2128]
    bsl = basel[:, ::STRIDE]                          # [B, S]

    in_maps = []
    for c in range(NC):
        fvec = np.zeros(NF, dtype=f)

        def fput(name, arr):
            a = np.asarray(arr, f).reshape(-1)
            fvec[FOF[name]:FOF[name] + a.size] = a

        fput('bsr', bsl[BL * c:BL * (c + 1)].reshape(NCH, 128).T)
        fput('BCn', bcn); fput('BCp', bcp)
        fput('g1s', g1); fput('b1s', b1)
        fput('ff1bs', np.asarray(ff1_b, f).reshape(2, 128).T)
        fput('g2s', np.asarray(g2, f).reshape(2, 128).T)
        fput('b2s', np.asarray(b2, f).reshape(2, 128).T)
        fput('ff2bs', ff2_b); fput('g3s', g3); fput('b3s', b3)
        fput('tvb', token_v_b)
        sl = slice(FC1L * c, FC1L * (c + 1))
        fput('fc1b', np.asarray(fc1_b, f)[sl])
        fput('bnf1g', np.asarray(bnf1_g, f)[sl]); fput('bnf1b', np.asarray(bnf1_b, f)[sl])
        fput('fc2b', fc2_b); fput('bnf2g', bnf2_g); fput('bnf2b', bnf2_b)
        fput('fc3b', fc3_b)

        pin = np.zeros(NTOT, dtype=BF)
        pin[0:NG] = gblob[NG * c:NG * (c + 1)]
        pin[F0:F0 + 2 * NF] = np.frombuffer(fvec.tobytes(), dtype=BF)
        pin[H0 + HOF['fc1ws']:H0 + HOF['fc1ws'] + 128 * 4 * FC1L] = \
            img128(np.asarray(fc1_w, f)[:, sl], 4, FC1L).astype(BF).reshape(-1)
        pin[H0 + HOF['fc2ws']:H0 + HOF['fc2ws'] + FC1L * FC2] = \
            np.asarray(fc2_w, f)[sl, :].astype(BF).reshape(-1)
        pin[H0 + HOF['xpad']:H0 + HOF['xpad'] + BL * LP] = \
            xpad[BL * c:BL * (c + 1)].astype(BF).reshape(-1)
        in_maps.append({"pin": pin.reshape(1, NTOT)})
    return in_maps


_NC_CACHE = None


def kernel(**inputs):
    global _NC_CACHE
    if _NC_CACHE is None:
        _NC_CACHE = _build()
    in_maps = _host_inputs(**inputs)
    res = run_bass_kernel_spmd(_NC_CACHE, in_maps, core_ids=list(range(NC)))
    return np.asarray(res.results[0]["out"], np.float32)


if __name__ == "__main__":
    import jax
    import reference
    cpu = jax.devices('cpu')[0]
    with jax.default_device(cpu):
        ins = {k: np.asarray(v) for k, v in reference.setup_inputs().items()}
        exp = np.asarray(reference.reference(**reference.setup_inputs()))
    out = kernel(**ins)
    err = np.abs(out - exp).max() / (np.abs(exp).max() + 1e-9)
    print("Relative error:", err)


# revision 20
# speedup vs baseline: 3.9949x; 3.9949x over previous
import sys
sys.path.insert(0, '/opt/trn_rl_repo')
import numpy as np
import ml_dtypes
from contextlib import ExitStack

import concourse.tile as tile
from concourse import bacc, mybir
import concourse.bass2jax as bass2jax
from concourse.ap import AP
from concourse.bass_utils import run_bass_kernel_spmd

BF = ml_dtypes.bfloat16
F32 = mybir.dt.float32
BF16 = mybir.dt.bfloat16
FP8 = mybir.dt.float8e4
AF = mybir.ActivationFunctionType
ALU = mybir.AluOpType
AX = mybir.AxisListType

TOKEN, STRIDE, NHEAD, EPS = 40, 4, 9, 1e-5
B, Bc, L = 32, 16, 2048
D, H, dk, S = 81, 9, 9, 512
FF, FC1, FC2, Y = 256, 512, 256, 4
NC = 8
BL = B // NC            # 4 local samples
TOKL = BL * S           # 2048 local tokens
NCH = TOKL // 128       # 16 chunks
CCH = (Bc * S) // 128   # 64 cali chunks
NTOT_BN = float(B * S)  # BN1-3 divisor
LP = L + 2 * TOKEN      # 2128 padded length
FC1L = FC1 // NC        # 64 fc1 cols per core

# ---------------- section offset tables ----------------
# Gather blob (bf16 elems, global offsets into the AllGathered flat [8*NG])
def _mk_alloc():
    st = [0]
    def a(n, align=2):
        st[0] = (st[0] + align - 1) // align * align
        o = st[0]; st[0] += n
        return o
    return a, st

_ga, _gst = _mk_alloc()
GOF = {
    'wqf':  _ga(D * D), 'wkf': _ga(D * D), 'wq2f': _ga(D * D),
    'wk2f': _ga(D * D), 'wv2f': _ga(D * D), 'wo1': _ga(D * D), 'wo2': _ga(D * D),
    'tvw':  _ga(D * D), 'beta1': _ga(D * D),
    'ff1w': _ga(D * FF),
    'ff2w0': _ga(128 * D), 'ff2w1': _ga(128 * D),
    'fc3w': _ga(128 * 2 * Y),
    # cvs/cms stored as fp8: half the bf16 footprint (offsets in bf16 units)
    'cvs':  _ga(128 * 4 * S // 2), 'cms': _ga(128 * 4 * S // 2),
    'pe':   _ga(128 * 4 * D), 'mask': _ga(128 * 4 * D),
    'cali': _ga(Bc * LP),
}
_GT = _gst[0]
NG = ((_GT + NC - 1) // NC + 1) // 2 * 2       # per-core slice len (even)
GTOT = NC * NG

# F section (f32 elems, offsets relative to F-section start)
_fa, _fst = _mk_alloc()
FOF = {
    'bsr':  _fa(128 * NCH),
    'BCn':  _fa(128 * 2), 'BCp': _fa(128 * 6),
    'g1s':  _fa(D), 'b1s': _fa(D),
    'ff1bs': _fa(128 * 2), 'g2s': _fa(128 * 2), 'b2s': _fa(128 * 2),
    'ff2bs': _fa(D), 'g3s': _fa(D), 'b3s': _fa(D),
    'tvb':  _fa(D),
    'fc1b': _fa(FC1L), 'bnf1g': _fa(FC1L), 'bnf1b': _fa(FC1L),
    'fc2b': _fa(FC2), 'bnf2g': _fa(FC2), 'bnf2b': _fa(FC2),
    'fc3b': _fa(Y),
}
NF = _fst[0]
F0 = NG                                        # bf16 offset of F section (even)
H0 = F0 + 2 * NF                               # bf16 offset of bf16 per-core section
_ha, _hst = _mk_alloc()
HOF = {'fc1ws': _ha(128 * 4 * FC1L), 'fc2ws': _ha(FC1L * FC2),
       'xpad': _ha(BL * LP)}
NH = _hst[0]
NTOT = H0 + NH
RG = [list(range(NC))]


def _build():
    nc = bacc.Bacc("TRN2", target_bir_lowering=False, debug=False,
                   enable_asserts=False, num_devices=NC)
    pin_d = nc.dram_tensor("pin", [1, NTOT], BF16, kind="ExternalInput")
    out_d = nc.dram_tensor("out", [B, Y], F32, kind="ExternalOutput")

    gin_d = nc.dram_tensor("gin", [1, NG], BF16)
    gath_d = nc.dram_tensor("gath", [NC, NG], BF16, addr_space="Shared")
    st1_in = nc.dram_tensor("st1_in", [1, 2 * D], F32)
    st1_out = nc.dram_tensor("st1_out", [1, 2 * D], F32, addr_space="Shared")
    st2_in = nc.dram_tensor("st2_in", [1, 2 * FF], F32)
    st2_out = nc.dram_tensor("st2_out", [1, 2 * FF], F32, addr_space="Shared")
    st3_in = nc.dram_tensor("st3_in", [1, 2 * D], F32)
    st3_out = nc.dram_tensor("st3_out", [1, 2 * D], F32, addr_space="Shared")
    fea_in = nc.dram_tensor("fea_in", [BL, S], F32)
    fea_out = nc.dram_tensor("fea_out", [B, S], F32, addr_space="Shared")
    ar2_in = nc.dram_tensor("ar2_in", [32, FC2], F32)
    ar2_out = nc.dram_tensor("ar2_out", [32, FC2], F32, addr_space="Shared")

    with tile.TileContext(nc) as tc, ExitStack() as ctx:
        pw = ctx.enter_context(tc.tile_pool(name="pw", bufs=1))     # weights/consts
        ps = ctx.enter_context(tc.tile_pool(name="ps", bufs=1))     # persistent acts
        pb = ctx.enter_context(tc.tile_pool(name="pb", bufs=2))     # rotating sbuf
        pp = ctx.enter_context(tc.tile_pool(name="pp", bufs=3, space="PSUM"))
        pacc = ctx.enter_context(tc.tile_pool(name="pacc", bufs=1, space="PSUM"))
        pt = ctx.enter_context(tc.tile_pool(name="pt", bufs=2, space="PSUM"))
        pscr = ctx.enter_context(tc.tile_pool(name="pscr", bufs=1))  # big scratch

        V = nc.vector; A_ = nc.scalar; T = nc.tensor; G = nc.gpsimd

        flat16 = pin_d[:].rearrange("a b -> (a b)")
        flat32 = flat16.bitcast(F32)
        gflat = gath_d[:].rearrange("k n -> (k n)")
        gh16 = gflat.tensor
        FB = F0 // 2   # f32-unit base of F section

        def gsl(name, n):
            return gflat[GOF[name]:GOF[name] + n]

        def fsl(name, n):
            return flat32[FB + FOF[name]:FB + FOF[name] + n]

        # ---- gather first: everything replicated rides the device fabric ----
        nc.sync.dma_start(gin_d[:], pin_d[:, 0:NG])
        nc.gpsimd.collective_compute("AllGather", ALU.bypass, replica_groups=RG,
                                     ins=[gin_d[:]], outs=[gath_d[:]])

        # ---- device-generated constants ----
        ident = pw.tile([128, 128], F32, tag="ident")
        V.memset(ident[:], 1.0)
        G.affine_select(ident[:], ident[:], [[-1, 128]], ALU.is_equal,
                        0.0, base=0, channel_multiplier=1)
        selT = pw.tile([dk, D * H], F32, tag="selT")
        V.memset(selT[:], 1.0)
        G.affine_select(selT[:], selT[:], [[-dk, H], [1, D]], ALU.is_equal,
                        0.0, base=0, channel_multiplier=-1)
        ones_r = pw.tile([1, 128], F32, tag="ones_r"); V.memset(ones_r[:], 1.0)
        ones128 = pw.tile([128, 1], F32, tag="ones128"); V.memset(ones128[:], 1.0)
        ones32 = pw.tile([32, 1], F32, tag="ones32"); V.memset(ones32[:], 1.0)
        epsc = pw.tile([128, 1], F32, tag="epsc"); V.memset(epsc[:], EPS)

        # ---- F-section loads (parallel with gather) ----
        def fload(name, p, cols):
            t = pw.tile([p, cols], F32, tag=name)
            nc.sync.dma_start(t[:], fsl(name, p * cols).rearrange("(p c) -> p c", c=cols))
            return t

        BCn = fload('BCn', 128, 2)
        BCp = fload('BCp', 128, 6)
        bsr = fload('bsr', 128, NCH)
        g1s = fload('g1s', D, 1); b1s = fload('b1s', D, 1)
        ff1bs = fload('ff1bs', 128, 2)
        g2s = fload('g2s', 128, 2); b2s = fload('b2s', 128, 2)
        ff2bs = fload('ff2bs', D, 1); g3s = fload('g3s', D, 1); b3s = fload('b3s', D, 1)
        tvb_r = fload('tvb', 1, D)
        fc1b = fload('fc1b', 1, FC1L)
        bnf1g = fload('bnf1g', 1, FC1L); bnf1b = fload('bnf1b', 1, FC1L)
        fc2b = fload('fc2b', 1, FC2)
        bnf2g = fload('bnf2g', 1, FC2); bnf2b = fload('bnf2b', 1, FC2)
        fc3b = fload('fc3b', 1, Y)

        # ---- head weights (per-core slices, bf16 -> f32) ----
        fc1ws = pw.tile([128, 4 * FC1L], F32, tag="fc1ws")
        fc2ws = pw.tile([FC1L, FC2], F32, tag="fc2ws")
        with tc.tile_pool(name="pld", bufs=1) as pld:
            t16 = pld.tile([128, 4 * FC1L], BF16, tag="fc1ws16")
            nc.sync.dma_start(t16[:], flat16[H0 + HOF['fc1ws']:H0 + HOF['fc1ws'] + 128 * 4 * FC1L]
                              .rearrange("(p c) -> p c", c=4 * FC1L))
            A_.activation(fc1ws[:], t16[:], AF.Copy)
            u16 = pld.tile([FC1L, FC2], BF16, tag="fc2ws16")
            nc.sync.dma_start(u16[:], flat16[H0 + HOF['fc2ws']:H0 + HOF['fc2ws'] + FC1L * FC2]
                              .rearrange("(p c) -> p c", c=FC2))
            A_.activation(fc2ws[:], u16[:], AF.Copy)

            # ---- gathered weights: bf16 -> f32 upcast ----
            def gload(name, p, cols, pool=pw):
                s16 = pld.tile([p, cols], BF16, tag=name + "16")
                nc.sync.dma_start(s16[:], gsl(name, p * cols).rearrange("(p c) -> p c", c=cols))
                t = pool.tile([p, cols], F32, tag=name)
                A_.activation(t[:], s16[:], AF.Copy)
                return t

            wqf = gload('wqf', D, D); wkf = gload('wkf', D, D)
            wq2f = gload('wq2f', D, D); wk2f = gload('wk2f', D, D); wv2f = gload('wv2f', D, D)
            wo1 = gload('wo1', D, D); wo2 = gload('wo2', D, D)
            tvw = gload('tvw', D, D); beta1 = gload('beta1', D, D)
            ff1w = gload('ff1w', D, FF)
            ff2w0 = gload('ff2w0', 128, D); ff2w1 = gload('ff2w1', 128, D)
            fc3w_s = gload('fc3w', 128, 2 * Y)
            pe_sb = gload('pe', 128, 4 * D)
            msk_sb = gload('mask', 128, 4 * D)

            # cvs/cms stay fp8 (feed matmuls directly; exp scale is tiny so fp8 noise is ~1e-3)
            g8 = gflat.bitcast(FP8)
            cvs = pw.tile([128, 4 * S], FP8, tag="cvs")
            nc.sync.dma_start(cvs[:], g8[2 * GOF['cvs']:2 * GOF['cvs'] + 128 * 4 * S]
                              .rearrange("(p c) -> p c", c=4 * S))
            cms = pw.tile([128, 4 * S], FP8, tag="cms")
            nc.sync.dma_start(cms[:], g8[2 * GOF['cms']:2 * GOF['cms'] + 128 * 4 * S]
                              .rearrange("(p c) -> p c", c=4 * S))

            # ---- E = exp(cwE * biasT) ----
            E = ps.tile([128, 4 * S], F32, tag="E")
            for ti in range(4):
                pbT = pp.tile([128, S], F32, tag="mm")
                for ui in range(4):
                    T.matmul(pbT[:], cvs[:, ui * S + ti * 128: ui * S + ti * 128 + 128],
                             cms[:, ui * S:(ui + 1) * S],
                             start=(ui == 0), stop=(ui == 3))
                A_.activation(E[:, ti * S:(ti + 1) * S], pbT[:], AF.Exp, scale=BCp[:, 1:2])

            # ---- tokenize x on device (overlapping-stride im2col DMA) ----
            h16 = flat16.tensor
            txT = ps.tile([D, TOKL], F32, tag="txT")
            for i in range(NCH):
                sc = i % 4
                b = i // 4
                x16 = pb.tile([128, D], BF16, tag="xtok16")
                nc.sync.dma_start(x16[:], AP(h16, H0 + HOF['xpad'] + LP * b + STRIDE * 128 * sc,
                                             [[STRIDE, 128], [1, D]]))
                r = pb.tile([128, D], F32, tag="ntok")
                V.tensor_scalar(r[:], x16[:], BCn[:, 0:1], BCn[:, 1:2], op0=ALU.mult, op1=ALU.add)
                V.tensor_tensor(r[:], r[:], msk_sb[:, sc * D:(sc + 1) * D], op=ALU.mult)
                V.tensor_tensor(r[:], r[:], pe_sb[:, sc * D:(sc + 1) * D], op=ALU.add)
                ptr = pt.tile([D, 128], F32, tag="tr")
                T.transpose(ptr[:], r[:], ident[:])
                A_.activation(txT[:, i * 128:(i + 1) * 128], ptr[:], AF.Copy)

        # ---- cali: tokenize (from gathered raw), project to cK/cV ----
        p2 = tc.tile_pool(name="p2", bufs=1)
        p2x = p2.__enter__()
        cK = p2x.tile([128, CCH * D], F32, tag="cK")
        cV = p2x.tile([128, CCH * D], F32, tag="cV")
        for j in range(CCH):
            sc = j % 4
            c = j // 4
            r16 = pb.tile([128, D], BF16, tag="ctok16")
            nc.sync.dma_start(r16[:], AP(gh16, GOF['cali'] + LP * c + STRIDE * 128 * sc,
                                         [[STRIDE, 128], [1, D]]))
            r = pb.tile([128, D], F32, tag="ntok")
            V.tensor_scalar(r[:], r16[:], BCn[:, 0:1], BCn[:, 1:2], op0=ALU.mult, op1=ALU.add)
            V.tensor_tensor(r[:], r[:], msk_sb[:, sc * D:(sc + 1) * D], op=ALU.mult)
            V.tensor_tensor(r[:], r[:], pe_sb[:, sc * D:(sc + 1) * D], op=ALU.add)
            ptr = pt.tile([D, 128], F32, tag="tr")
            T.transpose(ptr[:], r[:], ident[:])
            tcTs = pb.tile([D, 128], F32, tag="tcTs")
            A_.activation(tcTs[:], ptr[:], AF.Copy)
            pk = pp.tile([128, D], F32, tag="mm")
            T.matmul(pk[:], tcTs[:], wk2f[:])
            A_.activation(cK[:, j * D:(j + 1) * D], pk[:], AF.Copy)
            pv = pp.tile([128, D], F32, tag="mm")
            T.matmul(pv[:], tcTs[:], wv2f[:])
            A_.activation(cV[:, j * D:(j + 1) * D], pv[:], AF.Copy)

        # ---- Q2/K2/V2 token-major ----
        Q2 = p2x.tile([128, NCH * D], F32, tag="Q2")
        K2 = p2x.tile([128, NCH * D], F32, tag="K2")
        V2 = p2x.tile([128, NCH * D], F32, tag="V2")
        for i in range(NCH):
            lh = txT[:, i * 128:(i + 1) * 128]
            for w, dst in ((wq2f, Q2), (wk2f, K2), (wv2f, V2)):
                pj = pp.tile([128, D], F32, tag="mm")
                T.matmul(pj[:], lh, w[:])
                A_.activation(dst[:, i * D:(i + 1) * D], pj[:], AF.Copy)

        # ---- branch 2: banded batch attention (self + all cali columns) ----
        h2T = p2x.tile([D, TOKL], F32, tag="h2T")
        for i in range(NCH):
            sc = i % 4
            q = Q2[:, i * D:(i + 1) * D]
            sco = pb.tile([128, 17 * H], F32, tag="sco")
            prod = pb.tile([128, D], F32, tag="prod")
            V.tensor_tensor(prod[:], q, K2[:, i * D:(i + 1) * D], op=ALU.mult)
            V.tensor_reduce(sco[:, 0:H], prod[:].rearrange("p (h e) -> p h e", e=dk),
                            op=ALU.add, axis=AX.X)
            ckv = cK[:].rearrange("p (c f) -> p c f", f=4 * D)[:, :, sc * D:(sc + 1) * D]
            big = pb.tile([128, 16 * D], F32, tag="big")
            V.tensor_tensor(big[:].rearrange("p (c f) -> p c f", f=D),
                            ckv, q.unsqueeze(1).broadcast_to([128, 16, D]), op=ALU.mult)
            V.tensor_reduce(sco[:, H:].rearrange("p (c h) -> p c h", h=H),
                            big[:].rearrange("p (c h e) -> p c h e", h=H, e=dk),
                            op=ALU.add, axis=AX.X)
            esc = pb.tile([128, 17 * H], F32, tag="esc")
            A_.activation(esc[:], sco[:], AF.Exp, scale=1.0 / 3.0)
            den = pb.tile([128, H], F32, tag="den")
            V.tensor_reduce(den[:], esc[:].rearrange("p (k h) -> p h k", h=H),
                            op=ALU.add, axis=AX.X)
            rden = pb.tile([128, H], F32, tag="rden")
            V.reciprocal(rden[:], den[:])
            acc = pb.tile([128, D], F32, tag="acc")
            V.tensor_tensor(acc[:].rearrange("p (h e) -> p h e", e=dk),
                            V2[:, i * D:(i + 1) * D].rearrange("p (h e) -> p h e", e=dk),
                            esc[:, 0:H].unsqueeze(2).broadcast_to([128, H, dk]), op=ALU.mult)
            cvv = cV[:].rearrange("p (c f) -> p c f", f=4 * D)[:, :, sc * D:(sc + 1) * D]
            V.tensor_tensor(big[:].rearrange("p (c h e) -> p c h e", h=H, e=dk),
                            cvv.rearrange("p c (h e) -> p c h e", e=dk),
                            esc[:, H:].rearrange("p (c h) -> p c h", h=H).unsqueeze(3).broadcast_to([128, 16, H, dk]),
                            op=ALU.mult)
            accc = pb.tile([128, D], F32, tag="accc")
            V.tensor_reduce(accc[:], big[:].rearrange("p (c f) -> p f c", f=D),
                            op=ALU.add, axis=AX.X)
            V.tensor_tensor(acc[:], acc[:], accc[:], op=ALU.add)
            V.tensor_tensor(acc[:].rearrange("p (h e) -> p h e", e=dk),
                            acc[:].rearrange("p (h e) -> p h e", e=dk),
                            rden[:].unsqueeze(2).broadcast_to([128, H, dk]), op=ALU.mult)
            ptr = pt.tile([D, 128], F32, tag="tr")
            T.transpose(ptr[:], acc[:], ident[:])
            A_.activation(h2T[:, i * 128:(i + 1) * 128], ptr[:], AF.Copy)

        # ---- Wo2 -> h2o ; xh = tx + h2o ----
        h2oT = ps.tile([D, TOKL], F32, tag="h2oT")
        for n in range(4):
            p81 = pp.tile([D, S], F32, tag="mm")
            T.matmul(p81[:], wo2[:], h2T[:, n * S:(n + 1) * S])
            A_.activation(h2oT[:, n * S:(n + 1) * S], p81[:], AF.Copy)
        p2.__exit__(None, None, None)
        xhT = ps.tile([D, TOKL], F32, tag="xhT")
        V.tensor_tensor(xhT[:], txT[:], h2oT[:], op=ALU.add)

        # ---- branch 1 projections ----
        QsT = ps.tile([D, TOKL], F32, tag="QsT")
        KsT = ps.tile([D, TOKL], F32, tag="KsT")
        for n in range(4):
            p81 = pp.tile([D, S], F32, tag="mm")
            T.matmul(p81[:], wqf[:], xhT[:, n * S:(n + 1) * S])
            A_.activation(QsT[:, n * S:(n + 1) * S], p81[:], AF.Copy)
            p81b = pp.tile([D, S], F32, tag="mm")
            T.matmul(p81b[:], wkf[:], xhT[:, n * S:(n + 1) * S])
            A_.activation(KsT[:, n * S:(n + 1) * S], p81b[:], AF.Copy)
        Qstok = ps.tile([128, NCH * D], F32, tag="Qstok")
        for i in range(NCH):
            pj = pp.tile([128, D], F32, tag="mm")
            T.matmul(pj[:], xhT[:, i * 128:(i + 1) * 128], wqf[:])
            A_.activation(Qstok[:, i * D:(i + 1) * D], pj[:], AF.Copy)

        # ---- branch 1 attention ----
        attT = ps.tile([D, TOKL], F32, tag="attT")
        for b in range(BL):
            h1T_ps = pacc.tile([D, S], F32, tag="acc")
            for h in range(H):
                pqs = pp.tile([dk, S], F32, tag="mm")
                T.matmul(pqs[:], ident[0:D, h * dk:(h + 1) * dk], QsT[:, b * S:(b + 1) * S])
                Qsh = pb.tile([dk, S], F32, tag="Qsh")
                A_.activation(Qsh[:], pqs[:], AF.Copy)
                pks = pp.tile([dk, S], F32, tag="mm")
                T.matmul(pks[:], ident[0:D, h * dk:(h + 1) * dk], KsT[:, b * S:(b + 1) * S])
                Ksh = pb.tile([dk, S], F32, tag="Ksh")
                A_.activation(Ksh[:], pks[:], AF.Copy)
                hv = pacc.tile([dk, S], F32, tag="hv")
                hd = pacc.tile([1, S], F32, tag="hd")
                for k in range(4):
                    psc = pp.tile([128, S], F32, tag="mm")
                    T.matmul(psc[:], Ksh[:, k * 128:(k + 1) * 128], Qsh[:])
                    et = pb.tile([128, S], F32, tag="et")
                    A_.activation(et[:], psc[:], AF.Exp, scale=BCp[:, 0:1])
                    V.tensor_tensor(et[:], et[:], E[:, k * S:(k + 1) * S], op=ALU.mult)
                    T.matmul(hv[:], Qstok[:, (b * 4 + k) * D + h * dk:(b * 4 + k) * D + (h + 1) * dk],
                             et[:], start=(k == 0), stop=(k == 3))
                    T.matmul(hd[:], ones128[:], et[:], start=(k == 0), stop=(k == 3))
                rd = pb.tile([1, S], F32, tag="rec")
                V.reciprocal(rd[:], hd[:])
                prep = pp.tile([dk, S], F32, tag="mm")
                T.matmul(prep[:], ones_r[:, 0:dk], rd[:])
                reps = pb.tile([dk, S], F32, tag="reps")
                A_.activation(reps[:], prep[:], AF.Copy)
                vvn = pb.tile([dk, S], F32, tag="vvn")
                V.tensor_tensor(vvn[:], hv[:], reps[:], op=ALU.mult)
                T.matmul(h1T_ps[:], selT[:, h * D:(h + 1) * D], vvn[:],
                         start=(h == 0), stop=(h == 8))
            h1Tb = pb.tile([D, S], F32, tag="h1Tb")
            A_.activation(h1Tb[:], h1T_ps[:], AF.Copy)
            p81w = pp.tile([D, S], F32, tag="mm")
            T.matmul(p81w[:], wo1[:], h1Tb[:])
            V.tensor_scalar(attT[:, b * S:(b + 1) * S], p81w[:], BCp[:D, 3:4], None, op0=ALU.mult)
        V.tensor_scalar(h2oT[:], h2oT[:], BCp[:D, 2:3], None, op0=ALU.mult)
        V.tensor_tensor(attT[:], attT[:], h2oT[:], op=ALU.add)

        # ---- BN helper ----
        def bn_stats_ar(x_tiles, sti, sto, width):
            off = 0
            for t, p in x_tiles:
                s_ = pb.tile([p, 1], F32, tag="bnsum")
                V.tensor_reduce(s_[:], t[:], op=ALU.add, axis=AX.X)
                nc.sync.dma_start(sti[:, off:off + p].rearrange("a b -> (a b)"), s_[:])
                sq = pscr.tile([128, TOKL], F32, tag="scr")
                qs = pb.tile([p, 1], F32, tag="bnqs")
                A_.activation(sq[:p, 0:t.shape[1]], t[:], AF.Square)
                V.tensor_reduce(qs[:], sq[:p, 0:t.shape[1]], op=ALU.add, axis=AX.X)
                nc.sync.dma_start(sti[:, width + off:width + off + p].rearrange("a b -> (a b)"), qs[:])
                off += p
            nc.gpsimd.collective_compute("AllReduce", ALU.add, replica_groups=RG,
                                         ins=[sti[:]], outs=[sto[:]])
            out = []
            off = 0
            for t, p in x_tiles:
                st = pb.tile([p, 2], F32, tag="bnst")
                nc.sync.dma_start(st[:, 0:1], sto[:, off:off + p].rearrange("a b -> (a b)"))
                nc.sync.dma_start(st[:, 1:2], sto[:, width + off:width + off + p].rearrange("a b -> (a b)"))
                mean = pb.tile([p, 1], F32, tag="bnmean")
                V.tensor_scalar(mean[:], st[:, 0:1], 1.0 / NTOT_BN, None, op0=ALU.mult)
                var = pb.tile([p, 1], F32, tag="bnvar")
                V.tensor_scalar(var[:], st[:, 1:2], 1.0 / NTOT_BN, None, op0=ALU.mult)
                m2 = pb.tile([p, 1], F32, tag="bnm2")
                V.tensor_tensor(m2[:], mean[:], mean[:], op=ALU.mult)
                V.tensor_tensor(var[:], var[:], m2[:], op=ALU.subtract)
                std = pb.tile([p, 1], F32, tag="bnstd")
                A_.activation(std[:], var[:], AF.Sqrt, bias=epsc[:p, :])
                inv = pb.tile([p, 1], F32, tag="bninv")
                V.reciprocal(inv[:], std[:])
                out.append((mean, inv))
                off += p
            return out

        def bn_apply(dst, src, mean, inv, gg, bb, p):
            Ac = pb.tile([p, 1], F32, tag="bnA")
            V.tensor_tensor(Ac[:], inv[:], gg[:], op=ALU.mult)
            Bc_ = pb.tile([p, 1], F32, tag="bnB")
            V.tensor_tensor(Bc_[:], mean[:], Ac[:], op=ALU.mult)
            V.tensor_tensor(Bc_[:], bb[:], Bc_[:], op=ALU.subtract)
            V.tensor_scalar(dst[:], src[:], Ac[:], Bc_[:], op0=ALU.mult, op1=ALU.add)

        # ---- s1 = att + tx ; BN1 -> ma ----
        V.tensor_tensor(attT[:], attT[:], txT[:], op=ALU.add)
        (st1,) = bn_stats_ar([(attT, D)], st1_in, st1_out, D)
        pff = tc.tile_pool(name="pff", bufs=1)
        pffx = pff.__enter__()
        maT = pffx.tile([D, TOKL], F32, tag="maT")
        bn_apply(maT, attT, st1[0], st1[1], g1s, b1s, D)

        # ---- ff1 + BN2 ----
        f1a = pffx.tile([128, TOKL], F32, tag="f1a")
        f1b = pffx.tile([128, TOKL], F32, tag="f1b")
        for m, dst in ((0, f1a), (1, f1b)):
            for n in range(4):
                pw5 = pp.tile([128, S], F32, tag="mm")
                T.matmul(pw5[:], ff1w[:, m * 128:(m + 1) * 128], maT[:, n * S:(n + 1) * S])
                A_.activation(dst[:, n * S:(n + 1) * S], pw5[:], AF.Relu, bias=ff1bs[:, m:m + 1])
        stats2 = bn_stats_ar([(f1a, 128), (f1b, 128)], st2_in, st2_out, FF)
        bn_apply(f1a, f1a, stats2[0][0], stats2[0][1], g2s[:, 0:1], b2s[:, 0:1], 128)
        bn_apply(f1b, f1b, stats2[1][0], stats2[1][1], g2s[:, 1:2], b2s[:, 1:2], 128)

        # ---- ff2 ; s3 = ma + f2 ; BN3 -> res ----
        for n in range(4):
            pf2 = pp.tile([D, S], F32, tag="mm")
            T.matmul(pf2[:], ff2w0[:], f1a[:, n * S:(n + 1) * S], start=True, stop=False)
            T.matmul(pf2[:], ff2w1[:], f1b[:, n * S:(n + 1) * S], start=False, stop=True)
            f2c = pscr.tile([128, TOKL], F32, tag="scr")
            A_.activation(f2c[:D, 0:S], pf2[:], AF.Relu, bias=ff2bs[:])
            V.tensor_tensor(maT[:, n * S:(n + 1) * S], maT[:, n * S:(n + 1) * S], f2c[:D, 0:S], op=ALU.add)
        (st3,) = bn_stats_ar([(maT, D)], st3_in, st3_out, D)
        resT = pffx.tile([D, TOKL], F32, tag="resT")
        bn_apply(resT, maT, st3[0], st3[1], g3s, b3s, D)

        # ---- head: a, fea ----
        ptvb = pp.tile([128, D], F32, tag="mm")
        T.matmul(ptvb[:], ones_r[:], tvb_r[:])
        TVBr = pw.tile([128, D], F32, tag="TVBr")
        A_.activation(TVBr[:], ptvb[:], AF.Copy)
        feas = ps.tile([128, NCH], F32, tag="feas")
        for i in range(NCH):
            lh = resT[:, i * 128:(i + 1) * 128]
            ptv = pp.tile([128, D], F32, tag="mm")
            T.matmul(ptv[:], lh, tvw[:])
            tv = pb.tile([128, D], F32, tag="tv")
            V.tensor_tensor(tv[:], ptv[:], TVBr[:], op=ALU.add)
            pbt = pp.tile([128, D], F32, tag="mm")
            T.matmul(pbt[:], lh, beta1[:])
            eb = pb.tile([128, D], F32, tag="eb")
            ebs = pb.tile([128, 1], F32, tag="ebs")
            A_.activation(eb[:], pbt[:], AF.Exp)
            V.tensor_reduce(ebs[:], eb[:], op=ALU.add, axis=AX.X)
            rb = pb.tile([128, 1], F32, tag="rb")
            V.reciprocal(rb[:], ebs[:])
            V.tensor_tensor(tv[:], tv[:], eb[:], op=ALU.mult)
            av = pb.tile([128, 1], F32, tag="av")
            V.tensor_reduce(av[:], tv[:], op=ALU.add, axis=AX.X)
            V.tensor_tensor(av[:], av[:], rb[:], op=ALU.mult)
            V.tensor_scalar(av[:], av[:], BCp[:, 4:5], None, op0=ALU.mult)
            u = pb.tile([128, 1], F32, tag="u")
            V.tensor_scalar(u[:], bsr[:, i:i + 1], BCp[:, 5:6], None, op0=ALU.mult)
            V.tensor_tensor(feas[:, i:i + 1], av[:], u[:], op=ALU.add)
        pff.__exit__(None, None, None)
        nc.sync.dma_start(fea_in[:].rearrange("b s -> (b s)").rearrange("(j p) -> p j", p=128), feas[:])
        nc.gpsimd.collective_compute("AllGather", ALU.bypass, replica_groups=RG,
                                     ins=[fea_in[:]], outs=[fea_out[:]])

        # ---- sharded head: fc1 cols 64k..64k+64 local, fc2 partial + AllReduce ----
        with tc.tile_pool(name="ph", bufs=1) as ph:
            feaT = ph.tile([128, 4 * 32], F32, tag="feaT")
            for k_ in range(4):
                nc.sync.dma_start(feaT[:, k_ * 32:(k_ + 1) * 32],
                                  fea_out[:, k_ * 128:(k_ + 1) * 128].rearrange("b p -> p b"))
            ph1 = pacc.tile([32, FC1L], F32, tag="acc")
            for k_ in range(4):
                T.matmul(ph1[:], feaT[:, k_ * 32:(k_ + 1) * 32],
                         fc1ws[:, k_ * FC1L:(k_ + 1) * FC1L], start=(k_ == 0), stop=False)
            T.matmul(ph1[:], ones_r[:, 0:32], fc1b[:], start=False, stop=True)
            hh = ph.tile([32, FC1L], F32, tag="hh")
            A_.activation(hh[:], ph1[:], AF.Relu)

            def head_bn(xt, cols, gg, bb):
                pms = pp.tile([1, cols], F32, tag="mm")
                T.matmul(pms[:], ones32[:], xt[:])
                hsq = pscr.tile([128, TOKL], F32, tag="scr")
                V.tensor_tensor(hsq[0:32, 0:cols], xt[:], xt[:], op=ALU.mult)
                psq = pp.tile([1, cols], F32, tag="mm")
                T.matmul(psq[:], ones32[:], hsq[0:32, 0:cols])
                mean = ph.tile([1, FC2], F32, tag="hmean")
                V.tensor_scalar(mean[:, 0:cols], pms[:], 1.0 / 32.0, None, op0=ALU.mult)
                var = ph.tile([1, FC2], F32, tag="hvar")
                V.tensor_scalar(var[:, 0:cols], psq[:], 1.0 / 32.0, None, op0=ALU.mult)
                m2 = ph.tile([1, FC2], F32, tag="hm2")
                V.tensor_tensor(m2[:, 0:cols], mean[:, 0:cols], mean[:, 0:cols], op=ALU.mult)
                V.tensor_tensor(var[:, 0:cols], var[:, 0:cols], m2[:, 0:cols], op=ALU.subtract)
                std = ph.tile([1, FC2], F32, tag="hstd")
                A_.activation(std[:, 0:cols], var[:, 0:cols], AF.Sqrt, bias=epsc[0:1, :])
                inv = ph.tile([1, FC2], F32, tag="hinv")
                V.reciprocal(inv[:, 0:cols], std[:, 0:cols])
                Ar = ph.tile([1, FC2], F32, tag="hA")
                V.tensor_tensor(Ar[:, 0:cols], inv[:, 0:cols], gg[:], op=ALU.mult)
                Br = ph.tile([1, FC2], F32, tag="hB")
                V.tensor_tensor(Br[:, 0:cols], mean[:, 0:cols], Ar[:, 0:cols], op=ALU.mult)
                V.tensor_tensor(Br[:, 0:cols], bb[:], Br[:, 0:cols], op=ALU.subtract)
                pA = pp.tile([32, cols], F32, tag="mm")
                T.matmul(pA[:], ones_r[:, 0:32], Ar[:, 0:cols])
                pB = pp.tile([32, cols], F32, tag="mm")
                T.matmul(pB[:], ones_r[:, 0:32], Br[:, 0:cols])
                As_ = ph.tile([32, FC2], F32, tag="hAs")
                A_.activation(As_[:, 0:cols], pA[:], AF.Copy)
                Bs_ = ph.tile([32, FC2], F32, tag="hBs")
                A_.activation(Bs_[:, 0:cols], pB[:], AF.Copy)
                V.tensor_tensor(xt[:], xt[:], As_[:, 0:cols], op=ALU.mult)
                V.tensor_tensor(xt[:], xt[:], Bs_[:, 0:cols], op=ALU.add)

            head_bn(hh, FC1L, bnf1g, bnf1b)
            # fc2 partial: [32, 64] x [64, 256]
            ptk = pt.tile([FC1L, 32], F32, tag="tr")
            T.transpose(ptk[:], hh[:], ident[0:32, 0:32])
            hT = ph.tile([FC1L, 32], F32, tag="hT")
            A_.activation(hT[:], ptk[:], AF.Copy)
            ph2 = pacc.tile([32, FC2], F32, tag="acc")
            T.matmul(ph2[:], hT[:], fc2ws[:])
            p2s = ph.tile([32, FC2], F32, tag="p2s")
            A_.activation(p2s[:], ph2[:], AF.Copy)
            nc.sync.dma_start(ar2_in[:], p2s[:])
            nc.gpsimd.collective_compute("AllReduce", ALU.add, replica_groups=RG,
                                         ins=[ar2_in[:]], outs=[ar2_out[:]])
            gsum = ph.tile([32, FC2], F32, tag="gsum")
            nc.sync.dma_start(gsum[:], ar2_out[:])
            pbias = pp.tile([32, FC2], F32, tag="mm")
            T.matmul(pbias[:], ones_r[:, 0:32], fc2b[:])
            V.tensor_tensor(gsum[:], gsum[:], pbias[:], op=ALU.add)
            gh = ph.tile([32, FC2], F32, tag="gh")
            A_.activation(gh[:], gsum[:], AF.Relu)
            head_bn(gh, FC2, bnf2g, bnf2b)
            ph3 = pacc.tile([32, Y], F32, tag="acc")
            for k_ in range(2):
                ptk2 = pt.tile([128, 32], F32, tag="tr")
                T.transpose(ptk2[:], gh[:, k_ * 128:(k_ + 1) * 128], ident[0:32, 0:32])
                gTk = pb.tile([128, 32], F32, tag="gTk")
                A_.activation(gTk[:], ptk2[:], AF.Copy)
                T.matmul(ph3[:], gTk[:], fc3w_s[:, k_ * Y:(k_ + 1) * Y],
                         start=(k_ == 0), stop=False)
            T.matmul(ph3[:], ones_r[:, 0:32], fc3b[:], start=False, stop=True)
            osb = ph.tile([32, Y], F32, tag="osb")
            A_.activation(osb[:], ph3[:], AF.Tanh)
            nc.sync.dma_start(out_d[:], osb[:])
    nc.compile()
    return nc


# ---------------- cached PJRT dispatch ----------------
_PJRT_CACHE = {}
_orig_run_via_pjrt = bass2jax.run_bass_via_pjrt


def _cached_run_bass_via_pjrt(nc, in_maps, n_cores):
    try:
        import jax
        key = (id(nc), n_cores)
        ent = _PJRT_CACHE.get(key)
        if ent is None:
            bass2jax.install_neuronx_cc_hook()
            if nc.dbg_addr is not None:
                raise RuntimeError("dbg path not cached")
            partition_name = nc.partition_id_tensor.name if nc.partition_id_tensor else None
            in_names, out_names, out_avals, zero_shapes = [], [], [], []
            for alloc in nc.m.functions[0].allocations:
                if not isinstance(alloc, mybir.MemoryLocationSet):
                    continue
                name = alloc.memorylocations[0].name
                if alloc.kind == "ExternalInput":
                    if name != partition_name:
                        in_names.append(name)
                elif alloc.kind == "ExternalOutput":
                    out_names.append(name)
                    shape = tuple(alloc.tensor_shape)
                    dtype = mybir.dt.np(alloc.dtype)
                    out_avals.append(jax.core.ShapedArray(shape, dtype))
                    zero_shapes.append((shape, dtype))
            n_params = len(in_names)
            n_outs = len(out_avals)
            all_names = list(in_names) + out_names + ([partition_name] if partition_name else [])
            donate = tuple(range(n_params, n_params + n_outs))

            def _body(*args):
                operands = list(args)
                if partition_name is not None:
                    operands.append(bass2jax.partition_id_tensor())
                outs = bass2jax._bass_exec_p.bind(
                    *operands, out_avals=tuple(out_avals), in_names=tuple(all_names),
                    out_names=tuple(out_names), lowering_input_output_aliases=(),
                    sim_require_finite=True, sim_require_nnan=True, nc=nc)
                return tuple(outs)

            devices = jax.devices()[:n_cores]
            mesh = bass2jax.Mesh(np.asarray(devices), ("core",))
            in_specs = (bass2jax.PartitionSpec("core"),) * (n_params + n_outs)
            out_specs = (bass2jax.PartitionSpec("core"),) * n_outs
            sharded = jax.jit(
                bass2jax.shard_map(_body, mesh=mesh, in_specs=in_specs,
                                   out_specs=out_specs, check_rep=False),
                donate_argnums=donate, keep_unused=True)
            ent = (sharded, in_names, out_names, out_avals, zero_shapes)
            _PJRT_CACHE[key] = ent
        sharded, in_names, out_names, out_avals, zero_shapes = ent
        concat_in = [np.concatenate([np.asarray(m[nm]) for m in in_maps], axis=0)
                     for nm in in_names]
        concat_zeros = [np.zeros((n_cores * s[0], *s[1:]), d) for (s, d) in zero_shapes]
        out_arrs = sharded(*concat_in, *concat_zeros)
        return [
            {nm: np.asarray(out_arrs[i]).reshape(n_cores, *out_avals[i].shape)[c]
             for i, nm in enumerate(out_names)}
            for c in range(n_cores)
        ]
    except Exception:
        _PJRT_CACHE.pop((id(nc), n_cores), None)
        return _orig_run_via_pjrt(nc, in_maps, n_cores)


bass2jax.run_bass_via_pjrt = _cached_run_bass_via_pjrt


# ---------------- host packing ----------------
def _pe_mask_imgs():
    f = np.float32
    idx = np.arange(S)[:, None] * STRIDE + np.arange(D)[None, :]
    mask = ((idx >= TOKEN) & (idx < TOKEN + L)).astype(f)
    pos = np.arange(S, dtype=f)[:, None]
    div = np.exp(-np.log(f(10000.0)) * np.arange(0, D, 2, dtype=f) / D)
    ang = pos * div
    pe = np.zeros((S, D), dtype=f)
    pe[:, 0::2] = np.sin(ang)
    pe[:, 1::2] = np.cos(ang[:, : D // 2])
    img = lambda m: np.ascontiguousarray(m.reshape(4, 128, D).transpose(1, 0, 2)).reshape(128, 4 * D)
    return img(pe), img(mask)


_PE_IMG, _MASK_IMG = _pe_mask_imgs()


def _host_inputs(x, basel, cali_spec, Wq, Wk, Wq2, Wk2, Wv2, Cv, Wo1, Wo2,
                 corr_weight, h_weight, corr_map, g1, b1, ff1_w, ff1_b, g2, b2,
                 ff2_w, ff2_b, g3, b3, token_v_w, token_v_b, beta1, alpha1, alpha2,
                 fc1_w, fc1_b, bnf1_g, bnf1_b, fc2_w, fc2_b, bnf2_g, bnf2_b, fc3_w, fc3_b):
    f = np.float32
    x = np.asarray(x, f); basel = np.asarray(basel, f); cali_spec = np.asarray(cali_spec, f)

    # ---- gather blob (global, bf16) ----
    flat = lambda w: np.ascontiguousarray(np.asarray(w, f).transpose(1, 0, 2)).reshape(D, D)
    img128 = lambda m, c, w: np.ascontiguousarray(
        np.asarray(m, f).reshape(c, 128, w).transpose(1, 0, 2)).reshape(128, c * w)
    gbuf = np.zeros(GTOT, dtype=f)

    def gput(name, arr):
        a = np.asarray(arr, f).reshape(-1)
        gbuf[GOF[name]:GOF[name] + a.size] = a

    gput('wqf', flat(Wq)); gput('wkf', flat(Wk)); gput('wq2f', flat(Wq2))
    gput('wk2f', flat(Wk2)); gput('wv2f', flat(Wv2))
    gput('wo1', Wo1); gput('wo2', Wo2)
    gput('tvw', token_v_w); gput('beta1', beta1)
    gput('ff1w', ff1_w)
    gput('ff2w0', np.asarray(ff2_w, f)[0:128, :]); gput('ff2w1', np.asarray(ff2_w, f)[128:256, :])
    gput('fc3w', img128(fc3_w, 2, Y))
    gput('pe', _PE_IMG); gput('mask', _MASK_IMG)
    gput('cali', np.pad(cali_spec, ((0, 0), (TOKEN, TOKEN))))
    gblob = gbuf.astype(BF)
    np8 = ml_dtypes.float8_e4m3
    for nm, m in (('cvs', img128(Cv, 4, S)), ('cms', img128(np.asarray(corr_map, f).T, 4, S))):
        raw = np.frombuffer(m.astype(np8).tobytes(), dtype=BF)
        gblob[GOF[nm]:GOF[nm] + raw.size] = raw

    # ---- exact normalization scalars (reference semantics, host f32) ----
    xm = x[:, 20:-20].min()
    xs = np.abs((x[:, 20:-20] - xm).max())
    A = f(1.0) / xs
    Bn = -xm * A
    cw = f(np.asarray(corr_weight).reshape(-1)[0])
    hw = f(np.asarray(h_weight).reshape(-1)[0])
    a1 = f(np.asarray(alpha1).reshape(-1)[0])
    a2 = f(np.asarray(alpha2).reshape(-1)[0])
    bcn = np.broadcast_to(np.array([A, Bn], f), (128, 2))
    bcp = np.broadcast_to(np.array([(1.0 - cw) / 3.0, cw / np.sqrt(f(S)),
                                    hw, 1.0 - hw, a1, a2], f), (128, 6))

    xpad = np.pad(x, ((0, 0), (TOKEN, TOKEN)))        # [B, 2128]
    bsl = basel[:, ::STRIDE]                          # [B, S]

    in_maps = []
    for c in range(NC):
        fvec = np.zeros(NF, dtype=f)

        def fput(name, arr):
            a = np.asarray(arr, f).reshape(-1)
            fvec[FOF[name]:FOF[name] + a.size] = a

        fput('bsr', bsl[BL * c:BL * (c + 1)].reshape(NCH, 128).T)
        fput('BCn', bcn); fput('BCp', bcp)
        fput('g1s', g1); fput('b1s', b1)
        fput('ff1bs', np.asarray(ff1_b, f).reshape(2, 128).T)
        fput('g2s', np.asarray(g2, f).reshape(2, 128).T)
        fput('b2s', np.asarray(b2, f).reshape(2, 128).T)
        fput('ff2bs', ff2_b); fput('g3s', g3); fput('b3s', b3)
        fput('tvb', token_v_b)
        sl = slice(FC1L * c, FC1L * (c + 1))
        fput('fc1b', np.asarray(fc1_b, f)[sl])
        fput('bnf1g', np.asarray(bnf1_g, f)[sl]); fput('bnf1b', np.asarray(bnf1_b, f)[sl])
        fput('fc2b', fc2_b); fput('bnf2g', bnf2_g); fput('bnf2b', bnf2_b)
        fput('fc3b', fc3_b)

        pin = np.zeros(NTOT, dtype=BF)
        pin[0:NG] = gblob[NG * c:NG * (c + 1)]
        pin[F0:F0 + 2 * NF] = np.frombuffer(fvec.tobytes(), dtype=BF)
        pin[H0 + HOF['fc1ws']:H0 + HOF['fc1ws'] + 128 * 4 * FC1L] = \
            img128(np.asarray(fc1_w, f)[:, sl], 4, FC1L).astype(BF).reshape(-1)
        pin[H0 + HOF['fc2ws']:H0 + HOF['fc2ws'] + FC1L * FC2] = \
            np.asarray(fc2_w, f)[sl, :].astype(BF).reshape(-1)
        pin[H0 + HOF['xpad']:H0 + HOF['xpad'] + BL * LP] = \
            xpad[BL * c:BL * (c + 1)].astype(BF).reshape(-1)
        in_maps.append({"pin": pin.reshape(1, NTOT)})
    return in_maps


_NC_CACHE = None


def kernel(**inputs):
    global _NC_CACHE
    if _NC_CACHE is None:
        _NC_CACHE = _build()
    in_maps = _host_inputs(**inputs)
    res = run_bass_kernel_spmd(_NC_CACHE, in_maps, core_ids=list(range(NC)))
    return np.asarray(res.results[0]["out"], np.float32)


if __name__ == "__main__":
    import jax
    import reference
    cpu = jax.devices('cpu')[0]
    with jax.default_device(cpu):
        ins = {k: np.asarray(v) for k, v in reference.setup_inputs().items()}
        exp = np.asarray(reference.reference(**reference.setup_inputs()))
    out = kernel(**ins)
    err = np.abs(out - exp).max() / (np.abs(exp).max() + 1e-9)
    print("Relative error:", err)


# revision 22
# speedup vs baseline: 4.9112x; 1.2294x over previous
import sys
sys.path.insert(0, '/opt/trn_rl_repo')
import numpy as np
import ml_dtypes
from contextlib import ExitStack

import concourse.tile as tile
from concourse import bacc, mybir
import concourse.bass2jax as bass2jax
from concourse.ap import AP
from concourse.bass_utils import run_bass_kernel_spmd

BF = ml_dtypes.bfloat16
F32 = mybir.dt.float32
BF16 = mybir.dt.bfloat16
FP8 = mybir.dt.float8e4
AF = mybir.ActivationFunctionType
ALU = mybir.AluOpType
AX = mybir.AxisListType

TOKEN, STRIDE, NHEAD, EPS = 40, 4, 9, 1e-5
B, Bc, L = 32, 16, 2048
D, H, dk, S = 81, 9, 9, 512
FF, FC1, FC2, Y = 256, 512, 256, 4
NC = 8
BL = B // NC            # 4 local samples
TOKL = BL * S           # 2048 local tokens
NCH = TOKL // 128       # 16 chunks
CCH = (Bc * S) // 128   # 64 cali chunks
NTOT_BN = float(B * S)  # BN1-3 divisor
LP = L + 2 * TOKEN      # 2128 padded length
FC1L = FC1 // NC        # 64 fc1 cols per core

# ---------------- section offset tables ----------------
# Gather blob (bf16 elems, global offsets into the AllGathered flat [8*NG])
def _mk_alloc():
    st = [0]
    def a(n, align=2):
        st[0] = (st[0] + align - 1) // align * align
        o = st[0]; st[0] += n
        return o
    return a, st

_ga, _gst = _mk_alloc()
GOF = {
    'wqf':  _ga(D * D), 'wkf': _ga(D * D), 'wq2f': _ga(D * D),
    'wk2f': _ga(D * D), 'wv2f': _ga(D * D), 'wo1': _ga(D * D), 'wo2': _ga(D * D),
    'tvw':  _ga(D * D), 'beta1': _ga(D * D),
    'ff1w': _ga(D * FF),
    'ff2w0': _ga(128 * D), 'ff2w1': _ga(128 * D),
    'fc3w': _ga(128 * 2 * Y),
    # cvs/cms stored as fp8: half the bf16 footprint (offsets in bf16 units)
    'cvs':  _ga(128 * 4 * S // 2), 'cms': _ga(128 * 4 * S // 2),
    'pe':   _ga(128 * 4 * D), 'mask': _ga(128 * 4 * D),
    'cali': _ga(Bc * LP),
}
_GT = _gst[0]
NG = ((_GT + NC - 1) // NC + 1) // 2 * 2       # per-core slice len (even)
GTOT = NC * NG

# F section (f32 elems, offsets relative to F-section start)
_fa, _fst = _mk_alloc()
FOF = {
    'bsr':  _fa(128 * NCH),
    'BCn':  _fa(128 * 2), 'BCp': _fa(128 * 6),
    'g1s':  _fa(D), 'b1s': _fa(D),
    'ff1bs': _fa(128 * 2), 'g2s': _fa(128 * 2), 'b2s': _fa(128 * 2),
    'ff2bs': _fa(D), 'g3s': _fa(D), 'b3s': _fa(D),
    'tvb':  _fa(D),
    'fc1b': _fa(FC1L), 'bnf1g': _fa(FC1L), 'bnf1b': _fa(FC1L),
    'fc2b': _fa(FC2), 'bnf2g': _fa(FC2), 'bnf2b': _fa(FC2),
    'fc3b': _fa(Y),
}
NF = _fst[0]
F0 = NG                                        # bf16 offset of F section (even)
H0 = F0 + 2 * NF                               # bf16 offset of bf16 per-core section
_ha, _hst = _mk_alloc()
HOF = {'fc1ws': _ha(128 * 4 * FC1L), 'fc2ws': _ha(FC1L * FC2),
       'xpad': _ha(BL * LP)}
NH = _hst[0]
NTOT = H0 + NH
RG = [list(range(NC))]


def _build():
    nc = bacc.Bacc("TRN2", target_bir_lowering=False, debug=False,
                   enable_asserts=False, num_devices=NC)
    pin_d = nc.dram_tensor("pin", [1, NTOT], BF16, kind="ExternalInput")
    out_d = nc.dram_tensor("out", [B, Y], F32, kind="ExternalOutput")

    gin_d = nc.dram_tensor("gin", [1, NG], BF16)
    gath_d = nc.dram_tensor("gath", [NC, NG], BF16, addr_space="Shared")
    st1_in = nc.dram_tensor("st1_in", [1, 2 * D], F32)
    st1_out = nc.dram_tensor("st1_out", [1, 2 * D], F32, addr_space="Shared")
    st2_in = nc.dram_tensor("st2_in", [1, 2 * FF], F32)
    st2_out = nc.dram_tensor("st2_out", [1, 2 * FF], F32, addr_space="Shared")
    st3_in = nc.dram_tensor("st3_in", [1, 2 * D], F32)
    st3_out = nc.dram_tensor("st3_out", [1, 2 * D], F32, addr_space="Shared")
    fea_in = nc.dram_tensor("fea_in", [BL, S], F32)
    fea_out = nc.dram_tensor("fea_out", [B, S], F32, addr_space="Shared")
    ar2_in = nc.dram_tensor("ar2_in", [32, FC2], F32)
    ar2_out = nc.dram_tensor("ar2_out", [32, FC2], F32, addr_space="Shared")

    with tile.TileContext(nc) as tc, ExitStack() as ctx:
        pw = ctx.enter_context(tc.tile_pool(name="pw", bufs=1))     # weights/consts
        ps = ctx.enter_context(tc.tile_pool(name="ps", bufs=1))     # persistent acts
        pb = ctx.enter_context(tc.tile_pool(name="pb", bufs=2))     # rotating sbuf
        pp = ctx.enter_context(tc.tile_pool(name="pp", bufs=3, space="PSUM"))
        pacc = ctx.enter_context(tc.tile_pool(name="pacc", bufs=1, space="PSUM"))
        pt = ctx.enter_context(tc.tile_pool(name="pt", bufs=2, space="PSUM"))
        pscr = ctx.enter_context(tc.tile_pool(name="pscr", bufs=1))  # big scratch

        V = nc.vector; A_ = nc.scalar; T = nc.tensor; G = nc.gpsimd

        flat16 = pin_d[:].rearrange("a b -> (a b)")
        flat32 = flat16.bitcast(F32)
        gflat = gath_d[:].rearrange("k n -> (k n)")
        gh16 = gflat.tensor
        FB = F0 // 2   # f32-unit base of F section

        def gsl(name, n):
            return gflat[GOF[name]:GOF[name] + n]

        def fsl(name, n):
            return flat32[FB + FOF[name]:FB + FOF[name] + n]

        # ---- gather first: everything replicated rides the device fabric ----
        nc.sync.dma_start(gin_d[:], pin_d[:, 0:NG])
        nc.gpsimd.collective_compute("AllGather", ALU.bypass, replica_groups=RG,
                                     ins=[gin_d[:]], outs=[gath_d[:]])

        # ---- device-generated constants ----
        ident = pw.tile([128, 128], F32, tag="ident")
        V.memset(ident[:], 1.0)
        G.affine_select(ident[:], ident[:], [[-1, 128]], ALU.is_equal,
                        0.0, base=0, channel_multiplier=1)
        selT = pw.tile([dk, D * H], F32, tag="selT")
        V.memset(selT[:], 1.0)
        G.affine_select(selT[:], selT[:], [[-dk, H], [1, D]], ALU.is_equal,
                        0.0, base=0, channel_multiplier=-1)
        ones_r = pw.tile([1, 128], F32, tag="ones_r"); V.memset(ones_r[:], 1.0)
        ones128 = pw.tile([128, 1], F32, tag="ones128"); V.memset(ones128[:], 1.0)
        ones32 = pw.tile([32, 1], F32, tag="ones32"); V.memset(ones32[:], 1.0)
        epsc = pw.tile([128, 1], F32, tag="epsc"); V.memset(epsc[:], EPS)

        # ---- F-section loads (parallel with gather) ----
        def fload(name, p, cols):
            t = pw.tile([p, cols], F32, tag=name)
            nc.sync.dma_start(t[:], fsl(name, p * cols).rearrange("(p c) -> p c", c=cols))
            return t

        BCn = fload('BCn', 128, 2)
        BCp = fload('BCp', 128, 6)
        bsr = fload('bsr', 128, NCH)
        g1s = fload('g1s', D, 1); b1s = fload('b1s', D, 1)
        ff1bs = fload('ff1bs', 128, 2)
        g2s = fload('g2s', 128, 2); b2s = fload('b2s', 128, 2)
        ff2bs = fload('ff2bs', D, 1); g3s = fload('g3s', D, 1); b3s = fload('b3s', D, 1)
        tvb_r = fload('tvb', 1, D)
        fc1b = fload('fc1b', 1, FC1L)
        bnf1g = fload('bnf1g', 1, FC1L); bnf1b = fload('bnf1b', 1, FC1L)
        fc2b = fload('fc2b', 1, FC2)
        bnf2g = fload('bnf2g', 1, FC2); bnf2b = fload('bnf2b', 1, FC2)
        fc3b = fload('fc3b', 1, Y)

        # ---- head weights (per-core slices, bf16 -> f32) ----
        fc1ws = pw.tile([128, 4 * FC1L], F32, tag="fc1ws")
        fc2ws = pw.tile([FC1L, FC2], F32, tag="fc2ws")
        with tc.tile_pool(name="pld", bufs=1) as pld:
            t16 = pld.tile([128, 4 * FC1L], BF16, tag="fc1ws16")
            nc.sync.dma_start(t16[:], flat16[H0 + HOF['fc1ws']:H0 + HOF['fc1ws'] + 128 * 4 * FC1L]
                              .rearrange("(p c) -> p c", c=4 * FC1L))
            A_.activation(fc1ws[:], t16[:], AF.Copy)
            u16 = pld.tile([FC1L, FC2], BF16, tag="fc2ws16")
            nc.sync.dma_start(u16[:], flat16[H0 + HOF['fc2ws']:H0 + HOF['fc2ws'] + FC1L * FC2]
                              .rearrange("(p c) -> p c", c=FC2))
            A_.activation(fc2ws[:], u16[:], AF.Copy)

            # ---- gathered weights: bf16 -> f32 upcast ----
            def gload(name, p, cols, pool=pw):
                s16 = pld.tile([p, cols], BF16, tag=name + "16")
                nc.sync.dma_start(s16[:], gsl(name, p * cols).rearrange("(p c) -> p c", c=cols))
                t = pool.tile([p, cols], F32, tag=name)
                A_.activation(t[:], s16[:], AF.Copy)
                return t

            wqf = gload('wqf', D, D); wkf = gload('wkf', D, D)
            wq2f = gload('wq2f', D, D); wk2f = gload('wk2f', D, D); wv2f = gload('wv2f', D, D)
            wo1 = gload('wo1', D, D); wo2 = gload('wo2', D, D)
            tvw = gload('tvw', D, D); beta1 = gload('beta1', D, D)
            ff1w = gload('ff1w', D, FF)
            ff2w0 = gload('ff2w0', 128, D); ff2w1 = gload('ff2w1', 128, D)
            fc3w_s = gload('fc3w', 128, 2 * Y)
            pe_sb = gload('pe', 128, 4 * D)
            msk_sb = gload('mask', 128, 4 * D)

            # cvs/cms stay fp8 (feed matmuls directly; exp scale is tiny so fp8 noise is ~1e-3)
            g8 = gflat.bitcast(FP8)
            cvs = pw.tile([128, 4 * S], FP8, tag="cvs")
            nc.sync.dma_start(cvs[:], g8[2 * GOF['cvs']:2 * GOF['cvs'] + 128 * 4 * S]
                              .rearrange("(p c) -> p c", c=4 * S))
            cms = pw.tile([128, 4 * S], FP8, tag="cms")
            nc.sync.dma_start(cms[:], g8[2 * GOF['cms']:2 * GOF['cms'] + 128 * 4 * S]
                              .rearrange("(p c) -> p c", c=4 * S))

            # ---- E = exp(cwE * biasT) ----
            E = ps.tile([128, 4 * S], F32, tag="E")
            for ti in range(4):
                pbT = pp.tile([128, S], F32, tag="mm")
                for ui in range(4):
                    T.matmul(pbT[:], cvs[:, ui * S + ti * 128: ui * S + ti * 128 + 128],
                             cms[:, ui * S:(ui + 1) * S],
                             start=(ui == 0), stop=(ui == 3))
                A_.activation(E[:, ti * S:(ti + 1) * S], pbT[:], AF.Exp, scale=BCp[:, 1:2])

            # ---- tokenize x on device (overlapping-stride im2col DMA) ----
            h16 = flat16.tensor
            txT = ps.tile([D, TOKL], F32, tag="txT")
            for i in range(NCH):
                sc = i % 4
                b = i // 4
                x16 = pb.tile([128, D], BF16, tag="xtok16")
                nc.sync.dma_start(x16[:], AP(h16, H0 + HOF['xpad'] + LP * b + STRIDE * 128 * sc,
                                             [[STRIDE, 128], [1, D]]))
                r = pb.tile([128, D], F32, tag="ntok")
                V.tensor_scalar(r[:], x16[:], BCn[:, 0:1], BCn[:, 1:2], op0=ALU.mult, op1=ALU.add)
                V.tensor_tensor(r[:], r[:], msk_sb[:, sc * D:(sc + 1) * D], op=ALU.mult)
                V.tensor_tensor(r[:], r[:], pe_sb[:, sc * D:(sc + 1) * D], op=ALU.add)
                ptr = pt.tile([D, 128], F32, tag="tr")
                T.transpose(ptr[:], r[:], ident[:])
                A_.activation(txT[:, i * 128:(i + 1) * 128], ptr[:], AF.Copy)

        # ---- cali: tokenize (from gathered raw), project to cK/cV ----
        p2 = tc.tile_pool(name="p2", bufs=1)
        p2x = p2.__enter__()
        cK = p2x.tile([128, CCH * D], F32, tag="cK")
        cV = p2x.tile([128, CCH * D], F32, tag="cV")
        for j in range(CCH):
            sc = j % 4
            c = j // 4
            r16 = pb.tile([128, D], BF16, tag="ctok16")
            nc.sync.dma_start(r16[:], AP(gh16, GOF['cali'] + LP * c + STRIDE * 128 * sc,
                                         [[STRIDE, 128], [1, D]]))
            r = pb.tile([128, D], F32, tag="ntok")
            V.tensor_scalar(r[:], r16[:], BCn[:, 0:1], BCn[:, 1:2], op0=ALU.mult, op1=ALU.add)
            V.tensor_tensor(r[:], r[:], msk_sb[:, sc * D:(sc + 1) * D], op=ALU.mult)
            V.tensor_tensor(r[:], r[:], pe_sb[:, sc * D:(sc + 1) * D], op=ALU.add)
            ptr = pt.tile([D, 128], F32, tag="tr")
            T.transpose(ptr[:], r[:], ident[:])
            tcTs = pb.tile([D, 128], F32, tag="tcTs")
            A_.activation(tcTs[:], ptr[:], AF.Copy)
            pk = pp.tile([128, D], F32, tag="mm")
            T.matmul(pk[:], tcTs[:], wk2f[:])
            A_.activation(cK[:, j * D:(j + 1) * D], pk[:], AF.Copy)
            pv = pp.tile([128, D], F32, tag="mm")
            T.matmul(pv[:], tcTs[:], wv2f[:])
            A_.activation(cV[:, j * D:(j + 1) * D], pv[:], AF.Copy)

        # ---- Q2/K2/V2 token-major ----
        Q2 = p2x.tile([128, NCH * D], F32, tag="Q2")
        K2 = p2x.tile([128, NCH * D], F32, tag="K2")
        V2 = p2x.tile([128, NCH * D], F32, tag="V2")
        for i in range(NCH):
            lh = txT[:, i * 128:(i + 1) * 128]
            for w, dst in ((wq2f, Q2), (wk2f, K2), (wv2f, V2)):
                pj = pp.tile([128, D], F32, tag="mm")
                T.matmul(pj[:], lh, w[:])
                A_.activation(dst[:, i * D:(i + 1) * D], pj[:], AF.Copy)

        # ---- branch 2: banded batch attention (self + all cali columns) ----
        h2T = p2x.tile([D, TOKL], F32, tag="h2T")
        for i in range(NCH):
            sc = i % 4
            q = Q2[:, i * D:(i + 1) * D]
            sco = pb.tile([128, 17 * H], F32, tag="sco")
            prod = pb.tile([128, D], F32, tag="prod")
            V.tensor_tensor(prod[:], q, K2[:, i * D:(i + 1) * D], op=ALU.mult)
            V.tensor_reduce(sco[:, 0:H], prod[:].rearrange("p (h e) -> p h e", e=dk),
                            op=ALU.add, axis=AX.X)
            ckv = cK[:].rearrange("p (c f) -> p c f", f=4 * D)[:, :, sc * D:(sc + 1) * D]
            big = pb.tile([128, 16 * D], F32, tag="big")
            V.tensor_tensor(big[:].rearrange("p (c f) -> p c f", f=D),
                            ckv, q.unsqueeze(1).broadcast_to([128, 16, D]), op=ALU.mult)
            V.tensor_reduce(sco[:, H:].rearrange("p (c h) -> p c h", h=H),
                            big[:].rearrange("p (c h e) -> p c h e", h=H, e=dk),
                            op=ALU.add, axis=AX.X)
            esc = pb.tile([128, 17 * H], F32, tag="esc")
            A_.activation(esc[:], sco[:], AF.Exp, scale=1.0 / 3.0)
            den = pb.tile([128, H], F32, tag="den")
            V.tensor_reduce(den[:], esc[:].rearrange("p (k h) -> p h k", h=H),
                            op=ALU.add, axis=AX.X)
            rden = pb.tile([128, H], F32, tag="rden")
            V.reciprocal(rden[:], den[:])
            acc = pb.tile([128, D], F32, tag="acc")
            V.tensor_tensor(acc[:].rearrange("p (h e) -> p h e", e=dk),
                            V2[:, i * D:(i + 1) * D].rearrange("p (h e) -> p h e", e=dk),
                            esc[:, 0:H].unsqueeze(2).broadcast_to([128, H, dk]), op=ALU.mult)
            cvv = cV[:].rearrange("p (c f) -> p c f", f=4 * D)[:, :, sc * D:(sc + 1) * D]
            V.tensor_tensor(big[:].rearrange("p (c h e) -> p c h e", h=H, e=dk),
                            cvv.rearrange("p c (h e) -> p c h e", e=dk),
                            esc[:, H:].rearrange("p (c h) -> p c h", h=H).unsqueeze(3).broadcast_to([128, 16, H, dk]),
                            op=ALU.mult)
            accc = pb.tile([128, D], F32, tag="accc")
            V.tensor_reduce(accc[:], big[:].rearrange("p (c f) -> p f c", f=D),
                            op=ALU.add, axis=AX.X)
            V.tensor_tensor(acc[:], acc[:], accc[:], op=ALU.add)
            V.tensor_tensor(acc[:].rearrange("p (h e) -> p h e", e=dk),
                            acc[:].rearrange("p (h e) -> p h e", e=dk),
                            rden[:].unsqueeze(2).broadcast_to([128, H, dk]), op=ALU.mult)
            ptr = pt.tile([D, 128], F32, tag="tr")
            T.transpose(ptr[:], acc[:], ident[:])
            A_.activation(h2T[:, i * 128:(i + 1) * 128], ptr[:], AF.Copy)

        # ---- Wo2 -> h2o ; xh = tx + h2o ----
        h2oT = ps.tile([D, TOKL], F32, tag="h2oT")
        for n in range(4):
            p81 = pp.tile([D, S], F32, tag="mm")
            T.matmul(p81[:], wo2[:], h2T[:, n * S:(n + 1) * S])
            A_.activation(h2oT[:, n * S:(n + 1) * S], p81[:], AF.Copy)
        p2.__exit__(None, None, None)
        xhT = ps.tile([D, TOKL], F32, tag="xhT")
        V.tensor_tensor(xhT[:], txT[:], h2oT[:], op=ALU.add)

        # ---- branch 1 projections ----
        QsT = ps.tile([D, TOKL], F32, tag="QsT")
        KsT = ps.tile([D, TOKL], F32, tag="KsT")
        for n in range(4):
            p81 = pp.tile([D, S], F32, tag="mm")
            T.matmul(p81[:], wqf[:], xhT[:, n * S:(n + 1) * S])
            A_.activation(QsT[:, n * S:(n + 1) * S], p81[:], AF.Copy)
            p81b = pp.tile([D, S], F32, tag="mm")
            T.matmul(p81b[:], wkf[:], xhT[:, n * S:(n + 1) * S])
            A_.activation(KsT[:, n * S:(n + 1) * S], p81b[:], AF.Copy)
        Qstok = ps.tile([128, NCH * D], F32, tag="Qstok")
        for i in range(NCH):
            pj = pp.tile([128, D], F32, tag="mm")
            T.matmul(pj[:], xhT[:, i * 128:(i + 1) * 128], wqf[:])
            A_.activation(Qstok[:, i * D:(i + 1) * D], pj[:], AF.Copy)

        # ---- branch 1 attention ----
        attT = ps.tile([D, TOKL], F32, tag="attT")
        for b in range(BL):
            h1T_ps = pacc.tile([D, S], F32, tag="acc")
            for h in range(H):
                pqs = pp.tile([dk, S], F32, tag="mm")
                T.matmul(pqs[:], ident[0:D, h * dk:(h + 1) * dk], QsT[:, b * S:(b + 1) * S])
                Qsh = pb.tile([dk, S], F32, tag="Qsh")
                A_.activation(Qsh[:], pqs[:], AF.Copy)
                pks = pp.tile([dk, S], F32, tag="mm")
                T.matmul(pks[:], ident[0:D, h * dk:(h + 1) * dk], KsT[:, b * S:(b + 1) * S])
                Ksh = pb.tile([dk, S], F32, tag="Ksh")
                A_.activation(Ksh[:], pks[:], AF.Copy)
                hv = pacc.tile([dk, S], F32, tag="hv")
                hd = pacc.tile([1, S], F32, tag="hd")
                for k in range(4):
                    psc = pp.tile([128, S], F32, tag="mm")
                    T.matmul(psc[:], Ksh[:, k * 128:(k + 1) * 128], Qsh[:])
                    et = pb.tile([128, S], F32, tag="et")
                    A_.activation(et[:], psc[:], AF.Exp, scale=BCp[:, 0:1])
                    V.tensor_tensor(et[:], et[:], E[:, k * S:(k + 1) * S], op=ALU.mult)
                    T.matmul(hv[:], Qstok[:, (b * 4 + k) * D + h * dk:(b * 4 + k) * D + (h + 1) * dk],
                             et[:], start=(k == 0), stop=(k == 3))
                    T.matmul(hd[:], ones128[:], et[:], start=(k == 0), stop=(k == 3))
                rd = pb.tile([1, S], F32, tag="rec")
                V.reciprocal(rd[:], hd[:])
                prep = pp.tile([dk, S], F32, tag="mm")
                T.matmul(prep[:], ones_r[:, 0:dk], rd[:])
                reps = pb.tile([dk, S], F32, tag="reps")
                A_.activation(reps[:], prep[:], AF.Copy)
                vvn = pb.tile([dk, S], F32, tag="vvn")
                V.tensor_tensor(vvn[:], hv[:], reps[:], op=ALU.mult)
                T.matmul(h1T_ps[:], selT[:, h * D:(h + 1) * D], vvn[:],
                         start=(h == 0), stop=(h == 8))
            h1Tb = pb.tile([D, S], F32, tag="h1Tb")
            A_.activation(h1Tb[:], h1T_ps[:], AF.Copy)
            p81w = pp.tile([D, S], F32, tag="mm")
            T.matmul(p81w[:], wo1[:], h1Tb[:])
            V.tensor_scalar(attT[:, b * S:(b + 1) * S], p81w[:], BCp[:D, 3:4], None, op0=ALU.mult)
        V.tensor_scalar(h2oT[:], h2oT[:], BCp[:D, 2:3], None, op0=ALU.mult)
        V.tensor_tensor(attT[:], attT[:], h2oT[:], op=ALU.add)

        # ---- BN helper ----
        def bn_stats_ar(x_tiles, sti, sto, width):
            off = 0
            for t, p in x_tiles:
                s_ = pb.tile([p, 1], F32, tag="bnsum")
                V.tensor_reduce(s_[:], t[:], op=ALU.add, axis=AX.X)
                nc.sync.dma_start(sti[:, off:off + p].rearrange("a b -> (a b)"), s_[:])
                sq = pscr.tile([128, TOKL], F32, tag="scr")
                qs = pb.tile([p, 1], F32, tag="bnqs")
                A_.activation(sq[:p, 0:t.shape[1]], t[:], AF.Square)
                V.tensor_reduce(qs[:], sq[:p, 0:t.shape[1]], op=ALU.add, axis=AX.X)
                nc.sync.dma_start(sti[:, width + off:width + off + p].rearrange("a b -> (a b)"), qs[:])
                off += p
            nc.gpsimd.collective_compute("AllReduce", ALU.add, replica_groups=RG,
                                         ins=[sti[:]], outs=[sto[:]])
            out = []
            off = 0
            for t, p in x_tiles:
                st = pb.tile([p, 2], F32, tag="bnst")
                nc.sync.dma_start(st[:, 0:1], sto[:, off:off + p].rearrange("a b -> (a b)"))
                nc.sync.dma_start(st[:, 1:2], sto[:, width + off:width + off + p].rearrange("a b -> (a b)"))
                mean = pb.tile([p, 1], F32, tag="bnmean")
                V.tensor_scalar(mean[:], st[:, 0:1], 1.0 / NTOT_BN, None, op0=ALU.mult)
                var = pb.tile([p, 1], F32, tag="bnvar")
                V.tensor_scalar(var[:], st[:, 1:2], 1.0 / NTOT_BN, None, op0=ALU.mult)
                m2 = pb.tile([p, 1], F32, tag="bnm2")
                V.tensor_tensor(m2[:], mean[:], mean[:], op=ALU.mult)
                V.tensor_tensor(var[:], var[:], m2[:], op=ALU.subtract)
                std = pb.tile([p, 1], F32, tag="bnstd")
                A_.activation(std[:], var[:], AF.Sqrt, bias=epsc[:p, :])
                inv = pb.tile([p, 1], F32, tag="bninv")
                V.reciprocal(inv[:], std[:])
                out.append((mean, inv))
                off += p
            return out

        def bn_apply(dst, src, mean, inv, gg, bb, p):
            Ac = pb.tile([p, 1], F32, tag="bnA")
            V.tensor_tensor(Ac[:], inv[:], gg[:], op=ALU.mult)
            Bc_ = pb.tile([p, 1], F32, tag="bnB")
            V.tensor_tensor(Bc_[:], mean[:], Ac[:], op=ALU.mult)
            V.tensor_tensor(Bc_[:], bb[:], Bc_[:], op=ALU.subtract)
            V.tensor_scalar(dst[:], src[:], Ac[:], Bc_[:], op0=ALU.mult, op1=ALU.add)

        # ---- s1 = att + tx ; BN1 -> ma ----
        V.tensor_tensor(attT[:], attT[:], txT[:], op=ALU.add)
        (st1,) = bn_stats_ar([(attT, D)], st1_in, st1_out, D)
        pff = tc.tile_pool(name="pff", bufs=1)
        pffx = pff.__enter__()
        maT = pffx.tile([D, TOKL], F32, tag="maT")
        bn_apply(maT, attT, st1[0], st1[1], g1s, b1s, D)

        # ---- ff1 + BN2 ----
        f1a = pffx.tile([128, TOKL], F32, tag="f1a")
        f1b = pffx.tile([128, TOKL], F32, tag="f1b")
        for m, dst in ((0, f1a), (1, f1b)):
            for n in range(4):
                pw5 = pp.tile([128, S], F32, tag="mm")
                T.matmul(pw5[:], ff1w[:, m * 128:(m + 1) * 128], maT[:, n * S:(n + 1) * S])
                A_.activation(dst[:, n * S:(n + 1) * S], pw5[:], AF.Relu, bias=ff1bs[:, m:m + 1])
        stats2 = bn_stats_ar([(f1a, 128), (f1b, 128)], st2_in, st2_out, FF)
        bn_apply(f1a, f1a, stats2[0][0], stats2[0][1], g2s[:, 0:1], b2s[:, 0:1], 128)
        bn_apply(f1b, f1b, stats2[1][0], stats2[1][1], g2s[:, 1:2], b2s[:, 1:2], 128)

        # ---- ff2 ; s3 = ma + f2 ; BN3 -> res ----
        for n in range(4):
            pf2 = pp.tile([D, S], F32, tag="mm")
            T.matmul(pf2[:], ff2w0[:], f1a[:, n * S:(n + 1) * S], start=True, stop=False)
            T.matmul(pf2[:], ff2w1[:], f1b[:, n * S:(n + 1) * S], start=False, stop=True)
            f2c = pscr.tile([128, TOKL], F32, tag="scr")
            A_.activation(f2c[:D, 0:S], pf2[:], AF.Relu, bias=ff2bs[:])
            V.tensor_tensor(maT[:, n * S:(n + 1) * S], maT[:, n * S:(n + 1) * S], f2c[:D, 0:S], op=ALU.add)
        (st3,) = bn_stats_ar([(maT, D)], st3_in, st3_out, D)
        resT = pffx.tile([D, TOKL], F32, tag="resT")
        bn_apply(resT, maT, st3[0], st3[1], g3s, b3s, D)

        # ---- head: a, fea ----
        ptvb = pp.tile([128, D], F32, tag="mm")
        T.matmul(ptvb[:], ones_r[:], tvb_r[:])
        TVBr = pw.tile([128, D], F32, tag="TVBr")
        A_.activation(TVBr[:], ptvb[:], AF.Copy)
        feas = ps.tile([128, NCH], F32, tag="feas")
        for i in range(NCH):
            lh = resT[:, i * 128:(i + 1) * 128]
            ptv = pp.tile([128, D], F32, tag="mm")
            T.matmul(ptv[:], lh, tvw[:])
            tv = pb.tile([128, D], F32, tag="tv")
            V.tensor_tensor(tv[:], ptv[:], TVBr[:], op=ALU.add)
            pbt = pp.tile([128, D], F32, tag="mm")
            T.matmul(pbt[:], lh, beta1[:])
            eb = pb.tile([128, D], F32, tag="eb")
            ebs = pb.tile([128, 1], F32, tag="ebs")
            A_.activation(eb[:], pbt[:], AF.Exp)
            V.tensor_reduce(ebs[:], eb[:], op=ALU.add, axis=AX.X)
            rb = pb.tile([128, 1], F32, tag="rb")
            V.reciprocal(rb[:], ebs[:])
            V.tensor_tensor(tv[:], tv[:], eb[:], op=ALU.mult)
            av = pb.tile([128, 1], F32, tag="av")
            V.tensor_reduce(av[:], tv[:], op=ALU.add, axis=AX.X)
            V.tensor_tensor(av[:], av[:], rb[:], op=ALU.mult)
            V.tensor_scalar(av[:], av[:], BCp[:, 4:5], None, op0=ALU.mult)
            u = pb.tile([128, 1], F32, tag="u")
            V.tensor_scalar(u[:], bsr[:, i:i + 1], BCp[:, 5:6], None, op0=ALU.mult)
            V.tensor_tensor(feas[:, i:i + 1], av[:], u[:], op=ALU.add)
        pff.__exit__(None, None, None)
        nc.sync.dma_start(fea_in[:].rearrange("b s -> (b s)").rearrange("(j p) -> p j", p=128), feas[:])
        nc.gpsimd.collective_compute("AllGather", ALU.bypass, replica_groups=RG,
                                     ins=[fea_in[:]], outs=[fea_out[:]])

        # ---- sharded head: fc1 cols 64k..64k+64 local, fc2 partial + AllReduce ----
        with tc.tile_pool(name="ph", bufs=1) as ph:
            feaT = ph.tile([128, 4 * 32], F32, tag="feaT")
            for k_ in range(4):
                nc.sync.dma_start(feaT[:, k_ * 32:(k_ + 1) * 32],
                                  fea_out[:, k_ * 128:(k_ + 1) * 128].rearrange("b p -> p b"))
            ph1 = pacc.tile([32, FC1L], F32, tag="acc")
            for k_ in range(4):
                T.matmul(ph1[:], feaT[:, k_ * 32:(k_ + 1) * 32],
                         fc1ws[:, k_ * FC1L:(k_ + 1) * FC1L], start=(k_ == 0), stop=False)
            T.matmul(ph1[:], ones_r[:, 0:32], fc1b[:], start=False, stop=True)
            hh = ph.tile([32, FC1L], F32, tag="hh")
            A_.activation(hh[:], ph1[:], AF.Relu)

            def head_bn(xt, cols, gg, bb):
                pms = pp.tile([1, cols], F32, tag="mm")
                T.matmul(pms[:], ones32[:], xt[:])
                hsq = pscr.tile([128, TOKL], F32, tag="scr")
                V.tensor_tensor(hsq[0:32, 0:cols], xt[:], xt[:], op=ALU.mult)
                psq = pp.tile([1, cols], F32, tag="mm")
                T.matmul(psq[:], ones32[:], hsq[0:32, 0:cols])
                mean = ph.tile([1, FC2], F32, tag="hmean")
                V.tensor_scalar(mean[:, 0:cols], pms[:], 1.0 / 32.0, None, op0=ALU.mult)
                var = ph.tile([1, FC2], F32, tag="hvar")
                V.tensor_scalar(var[:, 0:cols], psq[:], 1.0 / 32.0, None, op0=ALU.mult)
                m2 = ph.tile([1, FC2], F32, tag="hm2")
                V.tensor_tensor(m2[:, 0:cols], mean[:, 0:cols], mean[:, 0:cols], op=ALU.mult)
                V.tensor_tensor(var[:, 0:cols], var[:, 0:cols], m2[:, 0:cols], op=ALU.subtract)
                std = ph.tile([1, FC2], F32, tag="hstd")
                A_.activation(std[:, 0:cols], var[:, 0:cols], AF.Sqrt, bias=epsc[0:1, :])
                inv = ph.tile([1, FC2], F32, tag="hinv")
                V.reciprocal(inv[:, 0:cols], std[:, 0:cols])
                Ar = ph.tile([1, FC2], F32, tag="hA")
                V.tensor_tensor(Ar[:, 0:cols], inv[:, 0:cols], gg[:], op=ALU.mult)
                Br = ph.tile([1, FC2], F32, tag="hB")
                V.tensor_tensor(Br[:, 0:cols], mean[:, 0:cols], Ar[:, 0:cols], op=ALU.mult)
                V.tensor_tensor(Br[:, 0:cols], bb[:], Br[:, 0:cols], op=ALU.subtract)
                pA = pp.tile([32, cols], F32, tag="mm")
                T.matmul(pA[:], ones_r[:, 0:32], Ar[:, 0:cols])
                pB = pp.tile([32, cols], F32, tag="mm")
                T.matmul(pB[:], ones_r[:, 0:32], Br[:, 0:cols])
                As_ = ph.tile([32, FC2], F32, tag="hAs")
                A_.activation(As_[:, 0:cols], pA[:], AF.Copy)
                Bs_ = ph.tile([32, FC2], F32, tag="hBs")
                A_.activation(Bs_[:, 0:cols], pB[:], AF.Copy)
                V.tensor_tensor(xt[:], xt[:], As_[:, 0:cols], op=ALU.mult)
                V.tensor_tensor(xt[:], xt[:], Bs_[:, 0:cols], op=ALU.add)

            head_bn(hh, FC1L, bnf1g, bnf1b)
            # fc2 partial: [32, 64] x [64, 256]
            ptk = pt.tile([FC1L, 32], F32, tag="tr")
            T.transpose(ptk[:], hh[:], ident[0:32, 0:32])
            hT = ph.tile([FC1L, 32], F32, tag="hT")
            A_.activation(hT[:], ptk[:], AF.Copy)
            ph2 = pacc.tile([32, FC2], F32, tag="acc")
            T.matmul(ph2[:], hT[:], fc2ws[:])
            p2s = ph.tile([32, FC2], F32, tag="p2s")
            A_.activation(p2s[:], ph2[:], AF.Copy)
            nc.sync.dma_start(ar2_in[:], p2s[:])
            nc.gpsimd.collective_compute("AllReduce", ALU.add, replica_groups=RG,
                                         ins=[ar2_in[:]], outs=[ar2_out[:]])
            gsum = ph.tile([32, FC2], F32, tag="gsum")
            nc.sync.dma_start(gsum[:], ar2_out[:])
            pbias = pp.tile([32, FC2], F32, tag="mm")
            T.matmul(pbias[:], ones_r[:, 0:32], fc2b[:])
            V.tensor_tensor(gsum[:], gsum[:], pbias[:], op=ALU.add)
            gh = ph.tile([32, FC2], F32, tag="gh")
            A_.activation(gh[:], gsum[:], AF.Relu)
            head_bn(gh, FC2, bnf2g, bnf2b)
            ph3 = pacc.tile([32, Y], F32, tag="acc")
            for k_ in range(2):
                ptk2 = pt.tile([128, 32], F32, tag="tr")
                T.transpose(ptk2[:], gh[:, k_ * 128:(k_ + 1) * 128], ident[0:32, 0:32])
                gTk = pb.tile([128, 32], F32, tag="gTk")
                A_.activation(gTk[:], ptk2[:], AF.Copy)
                T.matmul(ph3[:], gTk[:], fc3w_s[:, k_ * Y:(k_ + 1) * Y],
                         start=(k_ == 0), stop=False)
            T.matmul(ph3[:], ones_r[:, 0:32], fc3b[:], start=False, stop=True)
            osb = ph.tile([32, Y], F32, tag="osb")
            A_.activation(osb[:], ph3[:], AF.Tanh)
            nc.sync.dma_start(out_d[:], osb[:])
    nc.compile()
    return nc


# ---------------- cached PJRT dispatch ----------------
_PJRT_CACHE = {}
_orig_run_via_pjrt = bass2jax.run_bass_via_pjrt


def _cached_run_bass_via_pjrt(nc, in_maps, n_cores):
    try:
        import jax
        key = (id(nc), n_cores)
        ent = _PJRT_CACHE.get(key)
        if ent is None:
            bass2jax.install_neuronx_cc_hook()
            if nc.dbg_addr is not None:
                raise RuntimeError("dbg path not cached")
            partition_name = nc.partition_id_tensor.name if nc.partition_id_tensor else None
            in_names, out_names, out_avals, zero_shapes = [], [], [], []
            for alloc in nc.m.functions[0].allocations:
                if not isinstance(alloc, mybir.MemoryLocationSet):
                    continue
                name = alloc.memorylocations[0].name
                if alloc.kind == "ExternalInput":
                    if name != partition_name:
                        in_names.append(name)
                elif alloc.kind == "ExternalOutput":
                    out_names.append(name)
                    shape = tuple(alloc.tensor_shape)
                    dtype = mybir.dt.np(alloc.dtype)
                    out_avals.append(jax.core.ShapedArray(shape, dtype))
                    zero_shapes.append((shape, dtype))
            n_params = len(in_names)
            n_outs = len(out_avals)
            all_names = list(in_names) + out_names + ([partition_name] if partition_name else [])
            donate = tuple(range(n_params, n_params + n_outs))

            def _body(*args):
                operands = list(args)
                if partition_name is not None:
                    operands.append(bass2jax.partition_id_tensor())
                outs = bass2jax._bass_exec_p.bind(
                    *operands, out_avals=tuple(out_avals), in_names=tuple(all_names),
                    out_names=tuple(out_names), lowering_input_output_aliases=(),
                    sim_require_finite=True, sim_require_nnan=True, nc=nc)
                return tuple(outs)

            devices = jax.devices()[:n_cores]
            mesh = bass2jax.Mesh(np.asarray(devices), ("core",))
            in_specs = (bass2jax.PartitionSpec("core"),) * (n_params + n_outs)
            out_specs = (bass2jax.PartitionSpec("core"),) * n_outs
            sharded = jax.jit(
                bass2jax.shard_map(_body, mesh=mesh, in_specs=in_specs,
                                   out_specs=out_specs, check_rep=False),
                donate_argnums=donate, keep_unused=True)
            ent = (sharded, in_names, out_names, out_avals, zero_shapes)
            _PJRT_CACHE[key] = ent
        sharded, in_names, out_names, out_avals, zero_shapes = ent
        concat_in = [np.concatenate([np.asarray(m[nm]) for m in in_maps], axis=0)
                     for nm in in_names]
        concat_zeros = [np.zeros((n_cores * s[0], *s[1:]), d) for (s, d) in zero_shapes]
        out_arrs = sharded(*concat_in, *concat_zeros)
        return [
            {nm: np.asarray(out_arrs[i]).reshape(n_cores, *out_avals[i].shape)[c]
             for i, nm in enumerate(out_names)}
            for c in range(n_cores)
        ]
    except Exception:
        _PJRT_CACHE.pop((id(nc), n_cores), None)
        return _orig_run_via_pjrt(nc, in_maps, n_cores)


bass2jax.run_bass_via_pjrt = _cached_run_bass_via_pjrt


# ---------------- host packing ----------------
def _pe_mask_imgs():
    f = np.float32
    idx = np.arange(S)[:, None] * STRIDE + np.arange(D)[None, :]
    mask = ((idx >= TOKEN) & (idx < TOKEN + L)).astype(f)
    pos = np.arange(S, dtype=f)[:, None]
    div = np.exp(-np.log(f(10000.0)) * np.arange(0, D, 2, dtype=f) / D)
    ang = pos * div
    pe = np.zeros((S, D), dtype=f)
    pe[:, 0::2] = np.sin(ang)
    pe[:, 1::2] = np.cos(ang[:, : D // 2])
    img = lambda m: np.ascontiguousarray(m.reshape(4, 128, D).transpose(1, 0, 2)).reshape(128, 4 * D)
    return img(pe), img(mask)


_PE_IMG, _MASK_IMG = _pe_mask_imgs()


def _host_inputs(x, basel, cali_spec, Wq, Wk, Wq2, Wk2, Wv2, Cv, Wo1, Wo2,
                 corr_weight, h_weight, corr_map, g1, b1, ff1_w, ff1_b, g2, b2,
                 ff2_w, ff2_b, g3, b3, token_v_w, token_v_b, beta1, alpha1, alpha2,
                 fc1_w, fc1_b, bnf1_g, bnf1_b, fc2_w, fc2_b, bnf2_g, bnf2_b, fc3_w, fc3_b):
    f = np.float32
    x = np.asarray(x, f); basel = np.asarray(basel, f); cali_spec = np.asarray(cali_spec, f)

    # ---- gather blob (global, bf16) ----
    flat = lambda w: np.ascontiguousarray(np.asarray(w, f).transpose(1, 0, 2)).reshape(D, D)
    img128 = lambda m, c, w: np.ascontiguousarray(
        np.asarray(m, f).reshape(c, 128, w).transpose(1, 0, 2)).reshape(128, c * w)
    gbuf = np.zeros(GTOT, dtype=f)

    def gput(name, arr):
        a = np.asarray(arr, f).reshape(-1)
        gbuf[GOF[name]:GOF[name] + a.size] = a

    gput('wqf', flat(Wq)); gput('wkf', flat(Wk)); gput('wq2f', flat(Wq2))
    gput('wk2f', flat(Wk2)); gput('wv2f', flat(Wv2))
    gput('wo1', Wo1); gput('wo2', Wo2)
    gput('tvw', token_v_w); gput('beta1', beta1)
    gput('ff1w', ff1_w)
    gput('ff2w0', np.asarray(ff2_w, f)[0:128, :]); gput('ff2w1', np.asarray(ff2_w, f)[128:256, :])
    gput('fc3w', img128(fc3_w, 2, Y))
    gput('pe', _PE_IMG); gput('mask', _MASK_IMG)
    gput('cali', np.pad(cali_spec, ((0, 0), (TOKEN, TOKEN))))
    gblob = gbuf.astype(BF)
    np8 = ml_dtypes.float8_e4m3
    for nm, m in (('cvs', img128(Cv, 4, S)), ('cms', img128(np.asarray(corr_map, f).T, 4, S))):
        raw = np.frombuffer(m.astype(np8).tobytes(), dtype=BF)
        gblob[GOF[nm]:GOF[nm] + raw.size] = raw

    # ---- exact normalization scalars (reference semantics, host f32) ----
    xm = x[:, 20:-20].min()
    xs = np.abs((x[:, 20:-20] - xm).max())
    A = f(1.0) / xs
    Bn = -xm * A
    cw = f(np.asarray(corr_weight).reshape(-1)[0])
    hw = f(np.asarray(h_weight).reshape(-1)[0])
    a1 = f(np.asarray(alpha1).reshape(-1)[0])
    a2 = f(np.asarray(alpha2).reshape(-1)[0])
    bcn = np.broadcast_to(np.array([A, Bn], f), (128, 2))
    bcp = np.broadcast_to(np.array([(1.0 - cw) / 3.0, cw / np.sqrt(f(S)),
                                    hw, 1.0 - hw, a1, a2], f), (128, 6))

    xpad16 = np.pad(x, ((0, 0), (TOKEN, TOKEN))).astype(BF)   # [B, 2128]
    fc1w16 = np.asarray(fc1_w, f).astype(BF)
    fc2w16 = np.asarray(fc2_w, f).astype(BF)
    bsl = basel[:, ::STRIDE]                          # [B, S]

    in_maps = []
    for c in range(NC):
        fvec = np.zeros(NF, dtype=f)

        def fput(name, arr):
            a = np.asarray(arr, f).reshape(-1)
            fvec[FOF[name]:FOF[name] + a.size] = a

        fput('bsr', bsl[BL * c:BL * (c + 1)].reshape(NCH, 128).T)
        fput('BCn', bcn); fput('BCp', bcp)
        fput('g1s', g1); fput('b1s', b1)
        fput('ff1bs', np.asarray(ff1_b, f).reshape(2, 128).T)
        fput('g2s', np.asarray(g2, f).reshape(2, 128).T)
        fput('b2s', np.asarray(b2, f).reshape(2, 128).T)
        fput('ff2bs', ff2_b); fput('g3s', g3); fput('b3s', b3)
        fput('tvb', token_v_b)
        sl = slice(FC1L * c, FC1L * (c + 1))
        fput('fc1b', np.asarray(fc1_b, f)[sl])
        fput('bnf1g', np.asarray(bnf1_g, f)[sl]); fput('bnf1b', np.asarray(bnf1_b, f)[sl])
        fput('fc2b', fc2_b); fput('bnf2g', bnf2_g); fput('bnf2b', bnf2_b)
        fput('fc3b', fc3_b)

        pin = np.zeros(NTOT, dtype=BF)
        pin[0:NG] = gblob[NG * c:NG * (c + 1)]
        pin[F0:F0 + 2 * NF] = np.frombuffer(fvec.tobytes(), dtype=BF)
        pin[H0 + HOF['fc1ws']:H0 + HOF['fc1ws'] + 128 * 4 * FC1L] = \
            np.ascontiguousarray(fc1w16[:, sl].reshape(4, 128, FC1L).transpose(1, 0, 2)).reshape(-1)
        pin[H0 + HOF['fc2ws']:H0 + HOF['fc2ws'] + FC1L * FC2] = \
            fc2w16[sl, :].reshape(-1)
        pin[H0 + HOF['xpad']:H0 + HOF['xpad'] + BL * LP] = \
            xpad16[BL * c:BL * (c + 1)].reshape(-1)
        in_maps.append({"pin": pin.reshape(1, NTOT)})
    return in_maps


_NC_CACHE = None


def kernel(**inputs):
    global _NC_CACHE
    if _NC_CACHE is None:
        _NC_CACHE = _build()
    in_maps = _host_inputs(**inputs)
    res = run_bass_kernel_spmd(_NC_CACHE, in_maps, core_ids=list(range(NC)))
    return np.asarray(res.results[0]["out"], np.float32)


if __name__ == "__main__":
    import jax
    import reference
    cpu = jax.devices('cpu')[0]
    with jax.default_device(cpu):
        ins = {k: np.asarray(v) for k, v in reference.setup_inputs().items()}
        exp = np.asarray(reference.reference(**reference.setup_inputs()))
    out = kernel(**ins)
    err = np.abs(out - exp).max() / (np.abs(exp).max() + 1e-9)
    print("Relative error:", err)
